# revision 1
# baseline (speedup 1.0000x reference)
"""Trainium2 Bass kernel for nn_DelayLIFSNN.

Architecture (per reference):
  x (B, T0, J) -> delay_conv(w0,p0) -> BN(global batch stats) -> LIF
               -> delay_conv(w1,p1) -> BN -> LIF
               -> delay_conv(wr,pr) -> LI readout -> sum_t softmax_o -> (B, O)

Sharding: data-parallel over batch B across 8 cores (B_loc=32/core);
weights replicated; BN stats all-reduced ((128, 2*HT) f32 = 4KB each).

Conv = sum over K=25 taps of shifted matmuls accumulated in PSUM.
LIF = per-step scalar_tensor_tensor ops on DVE (sequential over time).
LI readout = tensor_tensor_scan. Softmax+time-sum via PE transpose + ones-matmul.

Activation layouts:
  x / spikes (conv rhs): [ch_tile][ch_part 128, t*B + b]   (DRAM: [CT,128,Tpad,B])
  conv out psum:         [out_part 128, t*B + b] per (ht, time-tile)
  y DRAM:                [HT, 128, T, B]
  LIF scan tiles:        [h_part 128, t*(HT*B) + ht*B + b]
  readout y3 DRAM:       [B*O, T3]  (rows b*O+o)
"""

import sys
import numpy as np

try:
    import concourse.bass as bass
except ImportError:  # grading env fallback
    sys.path.insert(0, "/opt/trn_rl_repo")
    import concourse.bass as bass

import concourse.mybir as mybir
import concourse.tile as tile
from contextlib import ExitStack
from concourse import bacc
from concourse.bass_utils import run_bass_kernel_spmd
from concourse.masks import make_identity

F32 = mybir.dt.float32
AF = mybir.ActivationFunctionType
OP = mybir.AluOpType


class Cfg:
    def __init__(self, T0=300, B_loc=32, J=140, H=512, O=20, K=25, n_cores=8,
                 BETA=0.95, THRESH=1.0, SIG=0.5, EPS=1e-5, NT=16, CH=48,
                 CHUNK_TT=6, dbg=False, max_phase=9, ablate=()):
        self.T0, self.B_loc, self.J, self.H, self.O, self.K = T0, B_loc, J, H, O, K
        self.n_cores = n_cores
        self.BETA, self.THRESH, self.SIG, self.EPS = BETA, THRESH, SIG, EPS
        self.LPAD, self.RPAD = K - 1, (K - 1) // 2
        self.PADT = self.LPAD + self.RPAD                      # 36
        self.T1 = T0 + self.RPAD                               # 312
        self.T2 = self.T1 + self.RPAD                          # 324
        self.T3 = self.T2 + self.RPAD                          # 336
        self.NT = NT                                           # out-steps per matmul tile
        self.CH = CH                                           # LIF chunk steps
        self.CHUNK_TT = CHUNK_TT                               # time-tiles per psum chunk
        self.HT = (H + 127) // 128                             # h tiles (4)
        self.B_tot = B_loc * n_cores
        self.dbg = dbg
        self.max_phase = max_phase
        self.ablate = set(ablate)
        self.J0 = min(J, 128)
        self.JL = J - self.J0                                  # leftover channels (12)


def split_tiles(total, size):
    out = []
    t = 0
    while t < total:
        n = min(size, total - t)
        out.append((t, n))
        t += n
    return out


def bc(ap, axis, count):
    """Insert a stride-0 (broadcast) axis at position `axis` of an AP."""
    dims = [list(d) for d in ap.ap]
    dims.insert(axis, [0, count])
    return bass.AP(tensor=ap.tensor, offset=ap.offset, ap=dims)


def build_kernel(cfg: Cfg):
    c = cfg
    B, HT, K, H, O = c.B_loc, c.HT, c.K, c.H, c.O
    nc = bacc.Bacc("TRN2", target_bir_lowering=False, debug=False,
                   num_devices=c.n_cores)

    tts1 = split_tiles(c.T1, c.NT)
    tts2 = split_tiles(c.T2, c.NT)
    tts3 = split_tiles(c.T3, c.NT)
    n1slots = len(tts1)
    n2slots = len(tts2)

    # ---- I/O ----
    xp = nc.dram_tensor("xp", [c.J, c.T0 + c.PADT, B], F32, kind="ExternalInput")
    w0a = nc.dram_tensor("w0a", [K, c.J0, H], F32, kind="ExternalInput")
    if c.JL:
        w0b = nc.dram_tensor("w0b", [K, c.JL, H], F32, kind="ExternalInput")
    w1t = nc.dram_tensor("w1t", [K, H, H], F32, kind="ExternalInput")
    wrt = nc.dram_tensor("wrt", [K, H, O], F32, kind="ExternalInput")
    g0m = nc.dram_tensor("g0m", [128, HT], F32, kind="ExternalInput")
    b0m = nc.dram_tensor("b0m", [128, HT], F32, kind="ExternalInput")
    g1m = nc.dram_tensor("g1m", [128, HT], F32, kind="ExternalInput")
    b1m = nc.dram_tensor("b1m", [128, HT], F32, kind="ExternalInput")
    selb = nc.dram_tensor("selb", [128, B], F32, kind="ExternalInput")
    out = nc.dram_tensor("out", [B, O], F32, kind="ExternalOutput")
    if c.dbg:
        d_y1 = nc.dram_tensor("d_y1", [HT, 128, c.T1, B], F32, kind="ExternalOutput")
        d_s1 = nc.dram_tensor("d_s1", [HT, 128, c.T1 + c.PADT, B], F32, kind="ExternalOutput")
        d_y2 = nc.dram_tensor("d_y2", [HT, 128, c.T2, B], F32, kind="ExternalOutput")
        d_y3 = nc.dram_tensor("d_y3", [O, c.T3, B], F32, kind="ExternalOutput")
        d_ac = nc.dram_tensor("d_ac", [128, 2 * HT + 2 * HT * B], F32, kind="ExternalOutput")
        d_st = nc.dram_tensor("d_st", [128, 2 * HT * len(tts1)], F32, kind="ExternalOutput")

    with tile.TileContext(nc) as tc, ExitStack() as ctx:
        dram = ctx.enter_context(tc.tile_pool(name="dram", bufs=1, space="DRAM"))
        y1d = dram.tile([HT, 128, c.T1, B], F32, name="y1d")
        s1d = dram.tile([HT, 128, c.T1 + c.PADT, B], F32, name="s1d")
        y2d = dram.tile([HT, 128, c.T2, B], F32, name="y2d")
        s2d = dram.tile([HT, 128, c.T2 + c.PADT, B], F32, name="s2d")
        y3d = dram.tile([O, c.T3, B], F32, name="y3d")
        cc_space = "Shared" if c.n_cores > 4 else "Local"
        cc1i = dram.tile([128, 2 * HT], F32, name="cc1i")
        cc1o = dram.tile([128, 2 * HT], F32, name="cc1o", addr_space=cc_space)
        cc2i = dram.tile([128, 2 * HT], F32, name="cc2i")
        cc2o = dram.tile([128, 2 * HT], F32, name="cc2o", addr_space=cc_space)

        glob = ctx.enter_context(tc.tile_pool(name="glob", bufs=1))

        # persistent small tiles
        sum1 = glob.tile([128, HT * n1slots], F32, name="sum1")
        sq1 = glob.tile([128, HT * n1slots], F32, name="sq1")
        sum2 = glob.tile([128, HT * n2slots], F32, name="sum2")
        sq2 = glob.tile([128, HT * n2slots], F32, name="sq2")
        gam0 = glob.tile([128, HT], F32, name="gam0")
        bet0 = glob.tile([128, HT], F32, name="bet0")
        gam1 = glob.tile([128, HT], F32, name="gam1")
        bet1 = glob.tile([128, HT], F32, name="bet1")
        if "nogb" not in c.ablate:
            nc.sync.dma_start(out=gam0, in_=g0m.ap())
            nc.sync.dma_start(out=bet0, in_=b0m.ap())
            nc.sync.dma_start(out=gam1, in_=g1m.ap())
            nc.sync.dma_start(out=bet1, in_=b1m.ap())
        A1 = glob.tile([128, HT], F32, name="A1")
        C1b = glob.tile([128, HT * B], F32, name="C1b")
        A2 = glob.tile([128, HT], F32, name="A2")
        C2b = glob.tile([128, HT * B], F32, name="C2b")
        zpad = glob.tile([128, c.LPAD * B], F32, name="zpad")
        nc.vector.memset(zpad, 0.0)

        # zero the pad regions of the spike dram buffers
        for sd, T in (() if "nozpad" in c.ablate else ((s1d, c.T1), (s2d, c.T2))):
            for ht in range(HT):
                nc.sync.dma_start(out=sd[ht, :, 0:c.LPAD, :],
                                  in_=zpad.rearrange("p (t b) -> p t b", b=B))
                nc.sync.dma_start(
                    out=sd[ht, :, T + c.LPAD:T + c.PADT, :],
                    in_=zpad.rearrange("p (t b) -> p t b", b=B)[:, :c.RPAD, :])

        # =============== Phase 1: conv1 (x -> y1) + stats ===============
        with ExitStack() as p1:
            psum = p1.enter_context(tc.tile_pool(name="psum1", bufs=8,
                                                  space="PSUM"))
            xpool = p1.enter_context(tc.tile_pool(name="xpool", bufs=1))
            wpool1 = p1.enter_context(tc.tile_pool(name="wpool1", bufs=1))
            stg1 = p1.enter_context(tc.tile_pool(name="stg1", bufs=3))

            T0p = c.T0 + c.PADT
            X0 = xpool.tile([c.J0, T0p * B], F32, name="X0")
            nc.sync.dma_start(out=X0.rearrange("p (t b) -> p t b", b=B),
                              in_=xp.ap()[:c.J0])
            W0 = wpool1.tile([c.J0, K * H], F32, name="W0")
            nc.sync.dma_start(out=W0.rearrange("p (k h) -> p k h", h=H),
                              in_=w0a.ap().rearrange("k p h -> p k h"))
            if c.JL:
                X1 = xpool.tile([c.JL, T0p * B], F32, name="X1")
                nc.sync.dma_start(out=X1.rearrange("p (t b) -> p t b", b=B),
                                  in_=xp.ap()[c.J0:])
                W1l = wpool1.tile([c.JL, K * H], F32, name="W1l")
                nc.sync.dma_start(out=W1l.rearrange("p (k h) -> p k h", h=H),
                                  in_=w0b.ap().rearrange("k p h -> p k h"))

            n_mm = K * (2 if c.JL else 1)
            zsrc = None
            if "nomm" in c.ablate:
                zsrc = wpool1.tile([128, c.NT * B], F32, name="zsrc")
                nc.vector.memset(zsrc, 0.0)
            for tti, (t0, nt) in enumerate(tts1):
                for ht in range(HT):
                    if "nomm" not in c.ablate:
                        ps = psum.tile([128, nt * B], F32, tag="cv1ps", name="ps1")
                        mi = 0
                        for kk in range(K):
                            nc.tensor.matmul(
                                ps, lhsT=W0[:, kk * H + ht * 128: kk * H + ht * 128 + 128],
                                rhs=X0[:, (t0 + kk) * B:(t0 + kk) * B + nt * B],
                                start=(mi == 0), stop=(mi == n_mm - 1))
                            mi += 1
                            if c.JL:
                                nc.tensor.matmul(
                                    ps,
                                    lhsT=W1l[:, kk * H + ht * 128: kk * H + ht * 128 + 128],
                                    rhs=X1[:, (t0 + kk) * B:(t0 + kk) * B + nt * B],
                                    start=(mi == 0), stop=(mi == n_mm - 1))
                                mi += 1
                        src = ps
                    else:
                        src = zsrc[:, :nt * B]
                    slot = ht * n1slots + tti
                    ystg = stg1.tile([128, nt * B], F32, tag="ystg", name="ystg")
                    if "nostats" not in c.ablate:
                        nc.scalar.activation(out=ystg, in_=src, func=AF.Copy,
                                             accum_out=sum1[:, slot:slot + 1])
                        ysq = stg1.tile([128, nt * B], F32, tag="ysq", name="ysq")
                        nc.scalar.activation(out=ysq, in_=src, func=AF.Square,
                                             accum_out=sq1[:, slot:slot + 1])
                    else:
                        nc.scalar.activation(out=ystg, in_=src, func=AF.Copy)
                    if "nostore" not in c.ablate:
                        nc.sync.dma_start(
                            out=y1d[ht, :, t0:t0 + nt, :],
                            in_=ystg.rearrange("p (t b) -> p t b", b=B))

        # =============== BN stats: allreduce + affine ===============
        def bn_affine(sumt, sqt, nslots, N, gam, bet, cci, cco, A, Cb, tagp):
            with ExitStack() as pb:
                sp = pb.enter_context(tc.tile_pool(name=f"bn{tagp}", bufs=1))
                ccs = sp.tile([128, 2 * HT], F32, name=f"ccs{tagp}")
                nc.vector.reduce_sum(
                    out=ccs[:, 0:HT],
                    in_=sumt.rearrange("p (h s) -> p h s", s=nslots),
                    axis=mybir.AxisListType.X)
                nc.vector.reduce_sum(
                    out=ccs[:, HT:2 * HT],
                    in_=sqt.rearrange("p (h s) -> p h s", s=nslots),
                    axis=mybir.AxisListType.X)
                nc.sync.dma_start(out=cci, in_=ccs)
                nc.gpsimd.collective_compute(
                    "AllReduce", OP.add,
                    replica_groups=[list(range(c.n_cores))],
                    ins=[cci], outs=[cco])
                gs = sp.tile([128, 2 * HT], F32, name=f"gs{tagp}")
                nc.sync.dma_start(out=gs, in_=cco)
                rN = float(1.0 / N)
                mu = sp.tile([128, HT], F32, name=f"mu{tagp}")
                nc.vector.tensor_scalar(mu, gs[:, 0:HT], rN, None, OP.mult)
                ex2 = sp.tile([128, HT], F32, name=f"ex2{tagp}")
                nc.vector.tensor_scalar(ex2, gs[:, HT:2 * HT], rN, None,
                                        OP.mult)
                var = sp.tile([128, HT], F32, name=f"var{tagp}")
                # var = ex2 - mu*mu ; then + eps
                nc.vector.scalar_tensor_tensor(out=var, in0=mu, scalar=1.0,
                                               in1=mu, op0=OP.mult, op1=OP.mult)
                nc.vector.tensor_sub(var, ex2, var)
                nc.vector.tensor_scalar_add(var, var, float(c.EPS))
                sv = sp.tile([128, HT], F32, name=f"sv{tagp}")
                nc.scalar.activation(out=sv, in_=var, func=AF.Sqrt)
                # one Newton step: s' = 0.5*(s + v/s)  (ACT sqrt is ~3e-6 approx)
                rs0 = sp.tile([128, HT], F32, name=f"rs0{tagp}")
                nc.vector.reciprocal(rs0, sv)
                t1 = sp.tile([128, HT], F32, name=f"t1{tagp}")
                nc.vector.tensor_mul(t1, var, rs0)
                nc.vector.tensor_add(sv, sv, t1)
                nc.vector.tensor_scalar(sv, sv, 0.5, None, OP.mult)
                rsv = sp.tile([128, HT], F32, name=f"rsv{tagp}")
                nc.vector.reciprocal(rsv, sv)
                nc.vector.tensor_mul(A, gam, rsv)
                # Cbias = bet - mu*A, broadcast over batch
                cb1 = sp.tile([128, HT], F32, name=f"cb1{tagp}")
                nc.vector.tensor_mul(cb1, mu, A)
                nc.vector.tensor_sub(cb1, bet, cb1)
                nc.vector.tensor_copy(
                    Cb.rearrange("p (h b) -> p h b", b=B), bc(cb1, 2, B))

        if c.max_phase >= 2:
            bn_affine(sum1, sq1, n1slots, c.T1 * c.B_tot, gam0, bet0,
                      cc1i, cc1o, A1, C1b, "1")

        # =============== LIF layer (generic) ===============
        def lif_layer(yd, sd, A, Cb, T, tag):
            with ExitStack() as pl:
                lp = pl.enter_context(tc.tile_pool(name=f"lif{tag}", bufs=2))
                up = pl.enter_context(tc.tile_pool(name=f"lifu{tag}", bufs=1))
                HTB = HT * B
                U = up.tile([128, HTB], F32, name=f"U{tag}")
                nc.vector.memset(U, 0.0)
                for (c0, cn) in split_tiles(T, c.CH):
                    ybufs = []
                    for ht in range(HT):
                        yb = lp.tile([128, cn * B], F32, tag=f"yb{ht}",
                                     name=f"yb{tag}")
                        nc.sync.dma_start(
                            out=yb.rearrange("p (t b) -> p t b", b=B),
                            in_=yd[ht, :, c0:c0 + cn, :])
                        ybufs.append(yb)
                    scn = lp.tile([128, cn * HTB], F32, tag="scn",
                                  name=f"scn{tag}")
                    scn3 = scn.rearrange("p (t x) -> p t x", x=HTB)
                    for ht in range(HT):
                        nc.vector.scalar_tensor_tensor(
                            out=scn3[:, :, ht * B:(ht + 1) * B],
                            in0=ybufs[ht].rearrange("p (t b) -> p t b", b=B),
                            scalar=A[:, ht:ht + 1],
                            in1=bc(Cb[:, ht * B:(ht + 1) * B], 1, cn),
                            op0=OP.mult, op1=OP.add)
                    S = lp.tile([128, cn * HTB], F32, tag="S", name=f"S{tag}")
                    for t in range(cn):
                        sl = slice(t * HTB, (t + 1) * HTB)
                        ut = lp.tile([128, HTB], F32, tag="ut", name=f"ut{tag}")
                        nc.vector.scalar_tensor_tensor(
                            out=ut, in0=U, scalar=float(c.BETA),
                            in1=scn[:, sl], op0=OP.mult, op1=OP.add)
                        nc.vector.tensor_scalar(
                            S[:, sl], ut, float(c.THRESH), None, OP.is_ge)
                        nc.vector.scalar_tensor_tensor(
                            out=U, in0=ut, scalar=float(c.THRESH), in1=ut,
                            op0=OP.is_lt, op1=OP.mult)
                    S3 = S.rearrange("p (t h b) -> p t h b", h=HT, b=B)
                    for ht in range(HT):
                        nc.sync.dma_start(
                            out=sd[ht, :, c.LPAD + c0:c.LPAD + c0 + cn, :],
                            in_=S3[:, :, ht, :])

        if c.max_phase >= 3:
            lif_layer(y1d, s1d, A1, C1b, c.T1, "1")

        # =============== conv from spikes (generic: layer 2 & readout) =====
        def conv_sp(sd, wsrc, M, tts, yd=None, sumt=None, sqt=None,
                    nslots=0, y3=None, tag=""):
            """y[o, t] = sum_{ct,k} W_k[ct]^T s[ct, t+k] (padded s)."""
            MT = (M + 127) // 128
            tchunks = split_tiles(len(tts), c.CHUNK_TT)
            with ExitStack() as pc:
                psum = pc.enter_context(tc.tile_pool(name=f"psum{tag}",
                                                     bufs=8, space="PSUM"))
                swp = pc.enter_context(tc.tile_pool(name=f"swin{tag}", bufs=2))
                wp = pc.enter_context(tc.tile_pool(name=f"w{tag}", bufs=3))
                sg = pc.enter_context(tc.tile_pool(name=f"stg{tag}", bufs=3))
                for (tci, ntt) in tchunks:
                    tt_group = tts[tci:tci + ntt]
                    w0_ = tt_group[0][0]
                    last_t0, last_nt = tt_group[-1]
                    winlen = (last_t0 + last_nt - 1 + K - 1) - w0_ + 1
                    swin = []
                    for ct in range(HT):
                        sw = swp.tile([128, winlen * B], F32, tag=f"sw{ct}",
                                      name=f"sw{tag}")
                        nc.sync.dma_start(
                            out=sw.rearrange("p (t b) -> p t b", b=B),
                            in_=sd[ct, :, w0_:w0_ + winlen, :])
                        swin.append(sw)
                    for ht in range(MT):
                        m0 = ht * 128
                        mtw = min(128, M - m0)
                        pss = [psum.tile([128, nt * B], F32, tag="cvps",
                                         name=f"ps{tag}")
                               for (t0, nt) in tt_group]
                        n_acc = HT * K
                        mi = 0
                        for ct in range(HT):
                            wt = wp.tile([128, K * mtw], F32, tag="wt",
                                         name=f"wt{tag}")
                            nc.sync.dma_start(
                                out=wt.rearrange("p (k m) -> p k m", m=mtw),
                                in_=wsrc.ap()[:, ct * 128:(ct + 1) * 128,
                                              m0:m0 + mtw].rearrange(
                                                  "k p m -> p k m"))
                            for kk in range(K):
                                lhsT = wt[:, kk * mtw:(kk + 1) * mtw]
                                st = (mi == 0)
                                sp_ = (mi == n_acc - 1)
                                for ti, (t0, nt) in enumerate(tt_group):
                                    off = (t0 - w0_ + kk) * B
                                    nc.tensor.matmul(
                                        pss[ti][:mtw], lhsT=lhsT,
                                        rhs=swin[ct][:, off:off + nt * B],
                                        start=st, stop=sp_)
                                mi += 1
                        for ti, (t0, nt) in enumerate(tt_group):
                            stg = sg.tile([128, nt * B], F32, tag="stg",
                                          name=f"stg{tag}")
                            if sumt is not None:
                                slot = ht * nslots + tci + ti
                                nc.scalar.activation(
                                    out=stg[:mtw], in_=pss[ti][:mtw],
                                    func=AF.Copy,
                                    accum_out=sumt[:, slot:slot + 1])
                                sqg = sg.tile([128, nt * B], F32, tag="sqg",
                                              name=f"sqg{tag}")
                                nc.scalar.activation(
                                    out=sqg[:mtw], in_=pss[ti][:mtw],
                                    func=AF.Square,
                                    accum_out=sqt[:, slot:slot + 1])
                            else:
                                nc.scalar.activation(out=stg[:mtw],
                                                     in_=pss[ti][:mtw],
                                                     func=AF.Copy)
                            if yd is not None:
                                nc.sync.dma_start(
                                    out=yd[ht, :, t0:t0 + nt, :],
                                    in_=stg.rearrange("p (t b) -> p t b", b=B))
                            else:  # readout: y3 is [O, T3, B]
                                nc.sync.dma_start(
                                    out=y3[m0:m0 + mtw, t0:t0 + nt, :],
                                    in_=stg[:mtw].rearrange(
                                        "p (t b) -> p t b", b=B))

        if c.max_phase >= 4:
            conv_sp(s1d, w1t, H, tts2, yd=y2d, sumt=sum2, sqt=sq2,
                    nslots=n2slots, tag="c2")
        if c.max_phase >= 5:
            bn_affine(sum2, sq2, n2slots, c.T2 * c.B_tot, gam1, bet1,
                      cc2i, cc2o, A2, C2b, "2")
        if c.max_phase >= 6:
            lif_layer(y2d, s2d, A2, C2b, c.T2, "2")
        if c.max_phase >= 7:
            conv_sp(s2d, wrt, O, tts3, y3=y3d, tag="c3")

        # =============== tail: LI scan, softmax over O, sum over t =========
        if c.max_phase < 8:
            with ExitStack() as pt:
                tp0 = pt.enter_context(tc.tile_pool(name="tail0", bufs=1))
                z = tp0.tile([B, O], F32, name="z")
                nc.vector.memset(z, 0.0)
                nc.sync.dma_start(out=out.ap(), in_=z)
        if c.max_phase >= 8:
            with ExitStack() as pt:
              psum = pt.enter_context(tc.tile_pool(name="psumt", bufs=1,
                                                   space="PSUM"))
              tp = pt.enter_context(tc.tile_pool(name="tail", bufs=1))
              tp2 = pt.enter_context(tc.tile_pool(name="tail2", bufs=3))
              TB = c.T3 * B
              Y3 = tp.tile([O, TB], F32, name="Y3")
              nc.sync.dma_start(out=Y3.rearrange("p (t b) -> p t b", b=B),
                                in_=y3d)
              beta_t = tp.tile([128, c.T3], F32, name="beta_t")
              nc.vector.memset(beta_t, float(c.BETA))
              idn = tp.tile([128, 128], F32, name="idn")
              make_identity(nc, idn)
              selbt = tp.tile([128, B], F32, name="selbt")
              nc.sync.dma_start(out=selbt, in_=selb.ap())
              us = tp.tile([O, TB], F32, name="us")
              # LI scan over t, one strided scan per batch column
              usv = us.rearrange("p (t b) -> p b t", b=B)
              y3v = Y3.rearrange("p (t b) -> p b t", b=B)
              for b in range(B):
                  nc.vector.tensor_tensor_scan(
                      out=usv[:, b, :], data0=beta_t[:O], data1=y3v[:, b, :],
                      initial=0.0, op0=OP.mult, op1=OP.add)
              # per-128-col blocks: transpose to (t*b, o), softmax over o, then
              # sum over t via selector matmul into (B, O)
              acc = psum.tile([B, O], F32, tag="accps", name="accps", bufs=1)
              blocks = split_tiles(TB, 128)
              for bi, (c0, cw) in enumerate(blocks):
                  pst = psum.tile([128, O], F32, tag="tpps", name="tpps", bufs=2)
                  nc.tensor.transpose(out=pst[:cw, :O],
                                      in_=us[:, c0:c0 + cw],
                                      identity=idn[:O, :O])
                  v = tp2.tile([128, O], F32, tag="v", name="v")
                  nc.scalar.copy(out=v[:cw], in_=pst[:cw, :O])
                  mx = tp2.tile([128, 1], F32, tag="mx", name="mx")
                  nc.vector.reduce_max(out=mx[:cw], in_=v[:cw],
                                       axis=mybir.AxisListType.X)
                  ev = tp2.tile([128, O], F32, tag="ev", name="ev")
                  nc.vector.tensor_scalar(ev[:cw], v[:cw], mx[:cw], None,
                                          OP.subtract)
                  pv = tp2.tile([128, O], F32, tag="pv", name="pv")
                  sm = tp2.tile([128, 1], F32, tag="sm", name="sm")
                  nc.scalar.activation(out=pv[:cw], in_=ev[:cw], func=AF.Exp,
                                       accum_out=sm[:cw])
                  rsm = tp2.tile([128, 1], F32, tag="rsm", name="rsm")
                  nc.vector.reciprocal(rsm[:cw], sm[:cw])
                  pn_t = tp2.tile([128, O], F32, tag="pnt", name="pnt")
                  nc.vector.tensor_scalar(pn_t[:cw], pv[:cw], rsm[:cw], None,
                                          OP.mult)
                  nc.tensor.matmul(
                      acc, lhsT=selbt[:cw], rhs=pn_t[:cw],
                      start=(bi == 0), stop=(bi == len(blocks) - 1),
                      skip_group_check=True)
              res = tp.tile([B, O], F32, name="res")
              nc.scalar.copy(out=res, in_=acc)
              nc.sync.dma_start(out=out.ap(), in_=res)
        if c.dbg:
            nc.sync.dma_start(out=d_y1.ap(), in_=y1d)
            nc.sync.dma_start(out=d_s1.ap(), in_=s1d)
            nc.sync.dma_start(out=d_y2.ap(), in_=y2d)
            nc.sync.dma_start(out=d_y3.ap(), in_=y3d)
            nc.sync.dma_start(out=d_ac.ap()[:, 0:HT], in_=A1)
            nc.sync.dma_start(out=d_ac.ap()[:, HT:2*HT], in_=A2)
            nc.sync.dma_start(out=d_ac.ap()[:, 2*HT:2*HT+HT*B], in_=C1b)
            nc.sync.dma_start(out=d_ac.ap()[:, 2*HT+HT*B:], in_=C2b)
            nc.sync.dma_start(out=d_st.ap()[:, 0:HT*n1slots], in_=sum1)
            nc.sync.dma_start(out=d_st.ap()[:, HT*n1slots:], in_=sq1)

    nc.compile()
    return nc


# ======================= host side =======================

def dcls_np(w, p, K, SIG):
    w = np.asarray(w, np.float32)
    p = np.asarray(p, np.float32)
    idx = np.arange(K, dtype=np.float32)
    d = idx[None, None, :] - np.float32(K // 2) - p[:, :, None]
    t = d / np.float32(SIG)
    g = np.exp(np.float32(-0.5) * t * t).astype(np.float32)
    g = g / (np.sum(g, axis=-1, keepdims=True, dtype=np.float32)
             + np.float32(1e-7))
    return (w[:, :, None] * g).astype(np.float32)


def make_in_maps(cfg: Cfg, x, w0, p0, g0, b0, w1, p1, g1, b1, wr, pr):
    c = cfg
    k0 = dcls_np(w0, p0, c.K, c.SIG)          # (H, J, K)
    k1 = dcls_np(w1, p1, c.K, c.SIG)          # (H, H, K)
    kr = dcls_np(wr, pr, c.K, c.SIG)          # (O, H, K)
    k0t = np.ascontiguousarray(k0.transpose(2, 1, 0))  # (K, J, H)
    k1t = np.ascontiguousarray(k1.transpose(2, 1, 0))  # (K, H, H)
    krt = np.ascontiguousarray(kr.transpose(2, 1, 0))  # (K, H, O)

    def chanmat(v):
        return np.ascontiguousarray(
            np.asarray(v, np.float32).reshape(c.HT, 128).T)

    nrep = max(1, 128 // c.B_loc)
    selb = np.ascontiguousarray(
        np.tile(np.eye(c.B_loc, dtype=np.float32), (nrep, 1)))[:128]
    shared = {
        "selb": selb,
        "w0a": np.ascontiguousarray(k0t[:, :c.J0, :]),
        "w1t": k1t, "wrt": krt,
        "g0m": chanmat(g0), "b0m": chanmat(b0),
        "g1m": chanmat(g1), "b1m": chanmat(b1),
    }
    if c.JL:
        shared["w0b"] = np.ascontiguousarray(k0t[:, c.J0:, :])

    in_maps = []
    x = np.asarray(x, np.float32)
    for ci in range(c.n_cores):
        xs = x[ci * c.B_loc:(ci + 1) * c.B_loc]          # (B_loc, T0, J)
        xpad = np.zeros((c.J, c.T0 + c.PADT, c.B_loc), np.float32)
        xpad[:, c.LPAD:c.LPAD + c.T0, :] = xs.transpose(2, 1, 0)
        m = dict(shared)
        m["xp"] = xpad
        in_maps.append(m)
    return in_maps


_CACHE = {}


def _get_nc(cfg: Cfg):
    key = (cfg.T0, cfg.B_loc, cfg.J, cfg.H, cfg.O, cfg.K, cfg.n_cores)
    if key not in _CACHE:
        _CACHE[key] = build_kernel(cfg)
    return _CACHE[key]


def run(cfg: Cfg, inputs, trace=False):
    nc = _get_nc(cfg)
    in_maps = make_in_maps(cfg, **inputs)
    res = run_bass_kernel_spmd(nc, in_maps, core_ids=list(range(cfg.n_cores)),
                               trace=trace)
    outs = [res.results[ci]["out"].reshape(cfg.B_loc, cfg.O)
            for ci in range(cfg.n_cores)]
    return np.concatenate(outs, axis=0), res


def kernel(**inputs):
    cfg = Cfg()
    out, _ = run(cfg, inputs)
    return out



# revision 17
# speedup vs baseline: 16.0064x; 16.0064x over previous
"""Trainium2 Bass kernel for nn_DelayLIFSNN.

Architecture (per reference):
  x (B, T0, J) -> delay_conv(w0,p0) -> BN(global batch stats) -> LIF
               -> delay_conv(w1,p1) -> BN -> LIF
               -> delay_conv(wr,pr) -> LI readout -> sum_t softmax_o -> (B, O)

Sharding: data-parallel over batch B across 8 cores (B_loc=32/core);
weights replicated; BN stats all-reduced ((128, 2*HT) f32 = 4KB each).

Host->device traffic is the bottleneck (axon tunnel ~60 MB/s), so the
Dcls gaussian delay kernels are expanded ON DEVICE from the raw (w, p)
parameters (2.7 MB/core) instead of shipping the K=25-expanded conv
kernels (34 MB/core). The jitted PJRT executable is cached across
kernel() calls.

Conv = sum over K=25 taps of shifted matmuls accumulated in PSUM.
LIF = per-step scalar_tensor_tensor ops on DVE (sequential over time).
LI readout = tensor_tensor_scan. Softmax+time-sum via PE transpose + ones-matmul.

Activation layouts:
  x / spikes (conv rhs): [ch_tile][ch_part 128, t*B + b]   (DRAM: [CT,128,Tpad,B])
  conv out psum:         [out_part 128, t*B + b] per (ht, time-tile)
  y DRAM:                [HT, 128, T, B]
  LIF scan tiles:        [h_part 128, t*(HT*B) + ht*B + b]
  readout y3 DRAM:       [B*O, T3]  (rows b*O+o)
"""

import sys
import numpy as np

try:
    import concourse.bass as bass
except ImportError:  # grading env fallback
    sys.path.insert(0, "/opt/trn_rl_repo")
    import concourse.bass as bass

import concourse.mybir as mybir
import concourse.tile as tile
from contextlib import ExitStack
from concourse import bacc
from concourse.bass_utils import run_bass_kernel_spmd
from concourse.masks import make_identity

F32 = mybir.dt.float32
BF16 = mybir.dt.bfloat16
U16 = mybir.dt.uint16
AF = mybir.ActivationFunctionType
OP = mybir.AluOpType
XSCALE = 65535.0  # u16 fixed-point scale for x in [0, 1]


class Cfg:
    def __init__(self, T0=300, B_loc=32, J=140, H=512, O=20, K=25, n_cores=8,
                 BETA=0.95, THRESH=1.0, SIG=0.5, EPS=1e-5, NT=16, CH=48,
                 CHUNK_TT=6, mm_bf16=False, x_u16=True, dbg=False,
                 max_phase=9, ablate=()):
        self.T0, self.B_loc, self.J, self.H, self.O, self.K = T0, B_loc, J, H, O, K
        self.n_cores = n_cores
        self.BETA, self.THRESH, self.SIG, self.EPS = BETA, THRESH, SIG, EPS
        self.LPAD, self.RPAD = K - 1, (K - 1) // 2
        self.PADT = self.LPAD + self.RPAD                      # 36
        self.T1 = T0 + self.RPAD                               # 312
        self.T2 = self.T1 + self.RPAD                          # 324
        self.T3 = self.T2 + self.RPAD                          # 336
        self.NT = NT                                           # out-steps per matmul tile
        self.CH = CH                                           # LIF chunk steps
        self.CHUNK_TT = CHUNK_TT                               # time-tiles per psum chunk
        self.HT = (H + 127) // 128                             # h tiles (4)
        self.B_tot = B_loc * n_cores
        self.mm_bf16 = mm_bf16                                 # matmul operand dtype
        self.MMDT = BF16 if mm_bf16 else F32
        self.x_u16 = x_u16                                     # ship x as u16 fixed point
        self.XDT = U16 if x_u16 else self.MMDT
        self.dbg = dbg
        self.max_phase = max_phase
        self.ablate = set(ablate)
        self.J0 = min(J, 128)
        self.JL = J - self.J0                                  # leftover channels (12)


def split_tiles(total, size):
    out = []
    t = 0
    while t < total:
        n = min(size, total - t)
        out.append((t, n))
        t += n
    return out


def bc(ap, axis, count):
    """Insert a stride-0 (broadcast) axis at position `axis` of an AP."""
    dims = [list(d) for d in ap.ap]
    dims.insert(axis, [0, count])
    return bass.AP(tensor=ap.tensor, offset=ap.offset, ap=dims)


def build_kernel(cfg: Cfg):
    c = cfg
    B, HT, K, H, O = c.B_loc, c.HT, c.K, c.H, c.O
    MMDT = c.MMDT
    nc = bacc.Bacc("TRN2", target_bir_lowering=False, debug=False,
                   num_devices=c.n_cores)

    tts1 = split_tiles(c.T1, c.NT)
    tts2 = split_tiles(c.T2, c.NT)
    tts3 = split_tiles(c.T3, c.NT)
    n1slots = len(tts1)
    n2slots = len(tts2)

    # ---- I/O ----
    # x, time-major transposed AND zero-padded on host (u16 zeros = 0.0)
    xp = nc.dram_tensor("xp", [c.J, c.T0 + c.PADT, B], c.XDT,
                        kind="ExternalInput")
    # raw Dcls params, transposed to [in_ch, out_ch] (expansion on device)
    w0T = nc.dram_tensor("w0T", [c.J, H], F32, kind="ExternalInput")
    p0T = nc.dram_tensor("p0T", [c.J, H], F32, kind="ExternalInput")
    w1T = nc.dram_tensor("w1T", [H, H], F32, kind="ExternalInput")
    p1T = nc.dram_tensor("p1T", [H, H], F32, kind="ExternalInput")
    wrT = nc.dram_tensor("wrT", [H, O], F32, kind="ExternalInput")
    prT = nc.dram_tensor("prT", [H, O], F32, kind="ExternalInput")
    g0m = nc.dram_tensor("g0m", [128, HT], F32, kind="ExternalInput")
    b0m = nc.dram_tensor("b0m", [128, HT], F32, kind="ExternalInput")
    g1m = nc.dram_tensor("g1m", [128, HT], F32, kind="ExternalInput")
    b1m = nc.dram_tensor("b1m", [128, HT], F32, kind="ExternalInput")
    selb = nc.dram_tensor("selb", [128, B], F32, kind="ExternalInput")
    out = nc.dram_tensor("out", [B, O], F32, kind="ExternalOutput")
    if c.dbg:
        d_y1 = nc.dram_tensor("d_y1", [HT, 128, c.T1, B], F32, kind="ExternalOutput")
        d_s1 = nc.dram_tensor("d_s1", [HT, 128, c.T1 + c.PADT, B], F32, kind="ExternalOutput")
        d_y2 = nc.dram_tensor("d_y2", [HT, 128, c.T2, B], F32, kind="ExternalOutput")
        d_y3 = nc.dram_tensor("d_y3", [O, c.T3, B], F32, kind="ExternalOutput")
        d_w = nc.dram_tensor("d_w", [K, c.J0, H], F32, kind="ExternalOutput")

    with tile.TileContext(nc) as tc, ExitStack() as ctx:
        dram = ctx.enter_context(tc.tile_pool(name="dram", bufs=1, space="DRAM"))
        # expanded Dcls kernels, produced on device
        w0a = dram.tile([K, c.J0, H], MMDT, name="w0a")
        w0b = dram.tile([K, c.JL, H], MMDT, name="w0b") if c.JL else None
        w1t = dram.tile([K, H, H], MMDT, name="w1t")
        wrt = dram.tile([K, H, O], MMDT, name="wrt")
        y1d = dram.tile([HT, 128, c.T1, B], F32, name="y1d")
        s1d = dram.tile([HT, 128, c.T1 + c.PADT, B], MMDT, name="s1d")
        y2d = dram.tile([HT, 128, c.T2, B], F32, name="y2d")
        s2d = dram.tile([HT, 128, c.T2 + c.PADT, B], MMDT, name="s2d")
        y3d = dram.tile([O, c.T3, B], F32, name="y3d")
        cc_space = "Shared" if c.n_cores > 4 else "Local"
        cc1i = dram.tile([128, 2 * HT], F32, name="cc1i")
        cc1o = dram.tile([128, 2 * HT], F32, name="cc1o", addr_space=cc_space)
        cc2i = dram.tile([128, 2 * HT], F32, name="cc2i")
        cc2o = dram.tile([128, 2 * HT], F32, name="cc2o", addr_space=cc_space)

        glob = ctx.enter_context(tc.tile_pool(name="glob", bufs=1))

        # persistent small tiles
        sum1 = glob.tile([128, HT * n1slots], F32, name="sum1")
        sq1 = glob.tile([128, HT * n1slots], F32, name="sq1")
        sum2 = glob.tile([128, HT * n2slots], F32, name="sum2")
        sq2 = glob.tile([128, HT * n2slots], F32, name="sq2")
        gam0 = glob.tile([128, HT], F32, name="gam0")
        bet0 = glob.tile([128, HT], F32, name="bet0")
        gam1 = glob.tile([128, HT], F32, name="gam1")
        bet1 = glob.tile([128, HT], F32, name="bet1")
        if "nogb" not in c.ablate:
            nc.sync.dma_start(out=gam0, in_=g0m.ap())
            nc.sync.dma_start(out=bet0, in_=b0m.ap())
            nc.sync.dma_start(out=gam1, in_=g1m.ap())
            nc.sync.dma_start(out=bet1, in_=b1m.ap())
        A1 = glob.tile([128, HT], F32, name="A1")
        C1b = glob.tile([128, HT * B], F32, name="C1b")
        A2 = glob.tile([128, HT], F32, name="A2")
        C2b = glob.tile([128, HT * B], F32, name="C2b")
        zpad = glob.tile([128, c.LPAD * B], MMDT, name="zpad")
        nc.vector.memset(zpad, 0.0)

        # zero the pad regions of the spike dram buffers
        for sd, T in (() if "nozpad" in c.ablate else ((s1d, c.T1), (s2d, c.T2))):
            for ht in range(HT):
                nc.sync.dma_start(out=sd[ht, :, 0:c.LPAD, :],
                                  in_=zpad.rearrange("p (t b) -> p t b", b=B))
                nc.sync.dma_start(
                    out=sd[ht, :, T + c.LPAD:T + c.PADT, :],
                    in_=zpad.rearrange("p (t b) -> p t b", b=B)[:, :c.RPAD, :])

        # =============== Phase 0: expand Dcls kernels on device ===========
        # g_k = exp(-0.5*((k - K//2 - p)/SIG)^2), normalized over k, * w.
        # SIG=0.5 -> exp(-2*(p - (k - K//2))^2).
        def expand(wsrc, psrc, r0, rp, M, dst, dstsl, tag):
            """rows r0:r0+rp of wsrc/psrc ([R, M] dram) -> dst[k][dstsl, :M]."""
            with ExitStack() as pe:
                ep = pe.enter_context(tc.tile_pool(name=f"exp{tag}", bufs=1))
                wp = ep.tile([rp, M], F32, name=f"wp{tag}")
                pp = ep.tile([rp, M], F32, name=f"pp{tag}")
                nc.sync.dma_start(out=wp, in_=wsrc.ap()[r0:r0 + rp, :])
                nc.sync.dma_start(out=pp, in_=psrc.ap()[r0:r0 + rp, :])
                eb = ep.tile([rp, K * M], F32, name=f"eb{tag}")
                ss = ep.tile([rp, M], F32, name=f"ss{tag}")
                dk = ep.tile([rp, M], F32, name=f"dk{tag}")
                sq = ep.tile([rp, M], F32, name=f"sq{tag}")
                inv2s2 = -1.0 / (2.0 * c.SIG * c.SIG)
                for k in range(K):
                    ek = eb[:, k * M:(k + 1) * M]
                    nc.vector.tensor_scalar(dk, pp, float(k - K // 2), None,
                                            OP.subtract)
                    nc.scalar.activation(out=sq, in_=dk, func=AF.Square)
                    nc.scalar.activation(out=ek, in_=sq, func=AF.Exp,
                                         scale=float(inv2s2))
                    if k == 0:
                        nc.vector.tensor_copy(ss, ek)
                    else:
                        nc.vector.tensor_add(ss, ss, ek)
                nc.vector.tensor_scalar_add(ss, ss, 1e-7)
                rn = ep.tile([rp, M], F32, name=f"rn{tag}")
                nc.vector.reciprocal(rn, ss)
                wn = ep.tile([rp, M], F32, name=f"wn{tag}")
                nc.vector.tensor_mul(wn, wp, rn)
                ok = ep.tile([rp, K * M], MMDT, name=f"ok{tag}")
                for k in range(K):
                    nc.vector.tensor_mul(ok[:, k * M:(k + 1) * M],
                                         eb[:, k * M:(k + 1) * M], wn)
                for k in range(K):
                    nc.sync.dma_start(out=dst[k, dstsl, :],
                                      in_=ok[:, k * M:(k + 1) * M])

        expand(w0T, p0T, 0, c.J0, H, w0a, slice(0, c.J0), "0a")
        if c.JL:
            expand(w0T, p0T, c.J0, c.JL, H, w0b, slice(0, c.JL), "0b")
        for ct in range(HT):
            expand(w1T, p1T, ct * 128, 128, H, w1t,
                   slice(ct * 128, (ct + 1) * 128), f"1_{ct}")
            expand(wrT, prT, ct * 128, 128, O, wrt,
                   slice(ct * 128, (ct + 1) * 128), f"r_{ct}")
        if c.dbg:
            nc.sync.dma_start(out=d_w.ap(), in_=w0a)

        # =============== Phase 1: conv1 (x -> y1) + stats ===============
        with ExitStack() as p1:
            psum = p1.enter_context(tc.tile_pool(name="psum1", bufs=8,
                                                  space="PSUM"))
            xpool = p1.enter_context(tc.tile_pool(name="xpool", bufs=1))

            T0p = c.T0 + c.PADT

            with ExitStack() as pxu:
                xup = (pxu.enter_context(tc.tile_pool(name="xup", bufs=1))
                       if c.x_u16 else None)

                def load_x(jp, j0):
                    """[jp, T0p*B] MMDT tile (host pre-padded), dequant."""
                    X = xpool.tile([jp, T0p * B], MMDT, name=f"X_{j0}")
                    if c.x_u16:
                        XU = xup.tile([jp, T0p * B], U16, name=f"XU_{j0}")
                        nc.sync.dma_start(
                            out=XU.rearrange("p (t b) -> p t b", b=B),
                            in_=xp.ap()[j0:j0 + jp])
                        nc.scalar.activation(out=X, in_=XU, func=AF.Copy,
                                             scale=float(1.0 / XSCALE))
                    else:
                        nc.sync.dma_start(
                            out=X.rearrange("p (t b) -> p t b", b=B),
                            in_=xp.ap()[j0:j0 + jp])
                    return X

                X0 = load_x(c.J0, 0)
                X1 = load_x(c.JL, c.J0) if c.JL else None

            wpool1 = p1.enter_context(tc.tile_pool(name="wpool1", bufs=1))
            stg1 = p1.enter_context(tc.tile_pool(name="stg1", bufs=3))
            W0 = wpool1.tile([c.J0, K * H], MMDT, name="W0")
            nc.sync.dma_start(out=W0.rearrange("p (k h) -> p k h", h=H),
                              in_=w0a.rearrange("k p h -> p k h"))
            if c.JL:
                W1l = wpool1.tile([c.JL, K * H], MMDT, name="W1l")
                nc.sync.dma_start(out=W1l.rearrange("p (k h) -> p k h", h=H),
                                  in_=w0b.rearrange("k p h -> p k h"))

            n_mm = K * (2 if c.JL else 1)
            zsrc = None
            if "nomm" in c.ablate:
                zsrc = wpool1.tile([128, c.NT * B], F32, name="zsrc")
                nc.vector.memset(zsrc, 0.0)
            for tti, (t0, nt) in enumerate(tts1):
                for ht in range(HT):
                    if "nomm" not in c.ablate:
                        ps = psum.tile([128, nt * B], F32, tag="cv1ps", name="ps1")
                        mi = 0
                        for kk in range(K):
                            nc.tensor.matmul(
                                ps, lhsT=W0[:, kk * H + ht * 128: kk * H + ht * 128 + 128],
                                rhs=X0[:, (t0 + kk) * B:(t0 + kk) * B + nt * B],
                                start=(mi == 0), stop=(mi == n_mm - 1))
                            mi += 1
                            if c.JL:
                                nc.tensor.matmul(
                                    ps,
                                    lhsT=W1l[:, kk * H + ht * 128: kk * H + ht * 128 + 128],
                                    rhs=X1[:, (t0 + kk) * B:(t0 + kk) * B + nt * B],
                                    start=(mi == 0), stop=(mi == n_mm - 1))
                                mi += 1
                        src = ps
                    else:
                        src = zsrc[:, :nt * B]
                    slot = ht * n1slots + tti
                    ystg = stg1.tile([128, nt * B], F32, tag="ystg", name="ystg")
                    if "nostats" not in c.ablate:
                        nc.scalar.activation(out=ystg, in_=src, func=AF.Copy,
                                             accum_out=sum1[:, slot:slot + 1])
                        ysq = stg1.tile([128, nt * B], F32, tag="ysq", name="ysq")
                        nc.scalar.activation(out=ysq, in_=src, func=AF.Square,
                                             accum_out=sq1[:, slot:slot + 1])
                    else:
                        nc.scalar.activation(out=ystg, in_=src, func=AF.Copy)
                    if "nostore" not in c.ablate:
                        nc.sync.dma_start(
                            out=y1d[ht, :, t0:t0 + nt, :],
                            in_=ystg.rearrange("p (t b) -> p t b", b=B))

        # =============== BN stats: allreduce + affine ===============
        def bn_affine(sumt, sqt, nslots, N, gam, bet, cci, cco, A, Cb, tagp):
            with ExitStack() as pb:
                sp = pb.enter_context(tc.tile_pool(name=f"bn{tagp}", bufs=1))
                ccs = sp.tile([128, 2 * HT], F32, name=f"ccs{tagp}")
                nc.vector.reduce_sum(
                    out=ccs[:, 0:HT],
                    in_=sumt.rearrange("p (h s) -> p h s", s=nslots),
                    axis=mybir.AxisListType.X)
                nc.vector.reduce_sum(
                    out=ccs[:, HT:2 * HT],
                    in_=sqt.rearrange("p (h s) -> p h s", s=nslots),
                    axis=mybir.AxisListType.X)
                nc.sync.dma_start(out=cci, in_=ccs)
                nc.gpsimd.collective_compute(
                    "AllReduce", OP.add,
                    replica_groups=[list(range(c.n_cores))],
                    ins=[cci], outs=[cco])
                gs = sp.tile([128, 2 * HT], F32, name=f"gs{tagp}")
                nc.sync.dma_start(out=gs, in_=cco)
                rN = float(1.0 / N)
                mu = sp.tile([128, HT], F32, name=f"mu{tagp}")
                nc.vector.tensor_scalar(mu, gs[:, 0:HT], rN, None, OP.mult)
                ex2 = sp.tile([128, HT], F32, name=f"ex2{tagp}")
                nc.vector.tensor_scalar(ex2, gs[:, HT:2 * HT], rN, None,
                                        OP.mult)
                var = sp.tile([128, HT], F32, name=f"var{tagp}")
                # var = ex2 - mu*mu ; then + eps
                nc.vector.scalar_tensor_tensor(out=var, in0=mu, scalar=1.0,
                                               in1=mu, op0=OP.mult, op1=OP.mult)
                nc.vector.tensor_sub(var, ex2, var)
                nc.vector.tensor_scalar_add(var, var, float(c.EPS))
                sv = sp.tile([128, HT], F32, name=f"sv{tagp}")
                nc.scalar.activation(out=sv, in_=var, func=AF.Sqrt)
                # one Newton step: s' = 0.5*(s + v/s)  (ACT sqrt is ~3e-6 approx)
                rs0 = sp.tile([128, HT], F32, name=f"rs0{tagp}")
                nc.vector.reciprocal(rs0, sv)
                t1 = sp.tile([128, HT], F32, name=f"t1{tagp}")
                nc.vector.tensor_mul(t1, var, rs0)
                nc.vector.tensor_add(sv, sv, t1)
                nc.vector.tensor_scalar(sv, sv, 0.5, None, OP.mult)
                rsv = sp.tile([128, HT], F32, name=f"rsv{tagp}")
                nc.vector.reciprocal(rsv, sv)
                nc.vector.tensor_mul(A, gam, rsv)
                # Cbias = bet - mu*A, broadcast over batch
                cb1 = sp.tile([128, HT], F32, name=f"cb1{tagp}")
                nc.vector.tensor_mul(cb1, mu, A)
                nc.vector.tensor_sub(cb1, bet, cb1)
                nc.vector.tensor_copy(
                    Cb.rearrange("p (h b) -> p h b", b=B), bc(cb1, 2, B))

        if c.max_phase >= 2:
            bn_affine(sum1, sq1, n1slots, c.T1 * c.B_tot, gam0, bet0,
                      cc1i, cc1o, A1, C1b, "1")

        # =============== LIF layer (generic) ===============
        def lif_layer(yd, sd, A, Cb, T, tag):
            with ExitStack() as pl:
                lp = pl.enter_context(tc.tile_pool(name=f"lif{tag}", bufs=2))
                up = pl.enter_context(tc.tile_pool(name=f"lifu{tag}", bufs=1))
                HTB = HT * B
                U = up.tile([128, HTB], F32, name=f"U{tag}")
                nc.vector.memset(U, 0.0)
                for (c0, cn) in split_tiles(T, c.CH):
                    ybufs = []
                    for ht in range(HT):
                        yb = lp.tile([128, cn * B], F32, tag=f"yb{ht}",
                                     name=f"yb{tag}")
                        nc.sync.dma_start(
                            out=yb.rearrange("p (t b) -> p t b", b=B),
                            in_=yd[ht, :, c0:c0 + cn, :])
                        ybufs.append(yb)
                    scn = lp.tile([128, cn * HTB], F32, tag="scn",
                                  name=f"scn{tag}")
                    scn3 = scn.rearrange("p (t x) -> p t x", x=HTB)
                    for ht in range(HT):
                        nc.vector.scalar_tensor_tensor(
                            out=scn3[:, :, ht * B:(ht + 1) * B],
                            in0=ybufs[ht].rearrange("p (t b) -> p t b", b=B),
                            scalar=A[:, ht:ht + 1],
                            in1=bc(Cb[:, ht * B:(ht + 1) * B], 1, cn),
                            op0=OP.mult, op1=OP.add)
                    S = lp.tile([128, cn * HTB], MMDT, tag="S", name=f"S{tag}")
                    for t in range(cn):
                        sl = slice(t * HTB, (t + 1) * HTB)
                        ut = lp.tile([128, HTB], F32, tag="ut", name=f"ut{tag}")
                        nc.vector.scalar_tensor_tensor(
                            out=ut, in0=U, scalar=float(c.BETA),
                            in1=scn[:, sl], op0=OP.mult, op1=OP.add)
                        nc.vector.tensor_scalar(
                            S[:, sl], ut, float(c.THRESH), None, OP.is_ge)
                        nc.vector.scalar_tensor_tensor(
                            out=U, in0=ut, scalar=float(c.THRESH), in1=ut,
                            op0=OP.is_lt, op1=OP.mult)
                    S3 = S.rearrange("p (t h b) -> p t h b", h=HT, b=B)
                    for ht in range(HT):
                        nc.sync.dma_start(
                            out=sd[ht, :, c.LPAD + c0:c.LPAD + c0 + cn, :],
                            in_=S3[:, :, ht, :])

        if c.max_phase >= 3:
            lif_layer(y1d, s1d, A1, C1b, c.T1, "1")

        # =============== conv from spikes (generic: layer 2 & readout) =====
        def conv_sp(sd, wsrc, M, tts, yd=None, sumt=None, sqt=None,
                    nslots=0, y3=None, tag=""):
            """y[o, t] = sum_{ct,k} W_k[ct]^T s[ct, t+k] (padded s)."""
            MT = (M + 127) // 128
            tchunks = split_tiles(len(tts), c.CHUNK_TT)
            with ExitStack() as pc:
                psum = pc.enter_context(tc.tile_pool(name=f"psum{tag}",
                                                     bufs=8, space="PSUM"))
                swp = pc.enter_context(tc.tile_pool(name=f"swin{tag}", bufs=2))
                wp = pc.enter_context(tc.tile_pool(name=f"w{tag}", bufs=3))
                sg = pc.enter_context(tc.tile_pool(name=f"stg{tag}", bufs=3))
                for (tci, ntt) in tchunks:
                    tt_group = tts[tci:tci + ntt]
                    w0_ = tt_group[0][0]
                    last_t0, last_nt = tt_group[-1]
                    winlen = (last_t0 + last_nt - 1 + K - 1) - w0_ + 1
                    swin = []
                    for ct in range(HT):
                        sw = swp.tile([128, winlen * B], MMDT, tag=f"sw{ct}",
                                      name=f"sw{tag}")
                        nc.sync.dma_start(
                            out=sw.rearrange("p (t b) -> p t b", b=B),
                            in_=sd[ct, :, w0_:w0_ + winlen, :])
                        swin.append(sw)
                    for ht in range(MT):
                        m0 = ht * 128
                        mtw = min(128, M - m0)
                        pss = [psum.tile([128, nt * B], F32, tag="cvps",
                                         name=f"ps{tag}")
                               for (t0, nt) in tt_group]
                        n_acc = HT * K
                        mi = 0
                        for ct in range(HT):
                            wt = wp.tile([128, K * mtw], MMDT, tag="wt",
                                         name=f"wt{tag}")
                            nc.sync.dma_start(
                                out=wt.rearrange("p (k m) -> p k m", m=mtw),
                                in_=wsrc[:, ct * 128:(ct + 1) * 128,
                                         m0:m0 + mtw].rearrange(
                                             "k p m -> p k m"))
                            for kk in range(K):
                                lhsT = wt[:, kk * mtw:(kk + 1) * mtw]
                                st = (mi == 0)
                                sp_ = (mi == n_acc - 1)
                                for ti, (t0, nt) in enumerate(tt_group):
                                    off = (t0 - w0_ + kk) * B
                                    nc.tensor.matmul(
                                        pss[ti][:mtw], lhsT=lhsT,
                                        rhs=swin[ct][:, off:off + nt * B],
                                        start=st, stop=sp_)
                                mi += 1
                        for ti, (t0, nt) in enumerate(tt_group):
                            stg = sg.tile([128, nt * B], F32, tag="stg",
                                          name=f"stg{tag}")
                            if sumt is not None:
                                slot = ht * nslots + tci + ti
                                nc.scalar.activation(
                                    out=stg[:mtw], in_=pss[ti][:mtw],
                                    func=AF.Copy,
                                    accum_out=sumt[:, slot:slot + 1])
                                sqg = sg.tile([128, nt * B], F32, tag="sqg",
                                              name=f"sqg{tag}")
                                nc.scalar.activation(
                                    out=sqg[:mtw], in_=pss[ti][:mtw],
                                    func=AF.Square,
                                    accum_out=sqt[:, slot:slot + 1])
                            else:
                                nc.scalar.activation(out=stg[:mtw],
                                                     in_=pss[ti][:mtw],
                                                     func=AF.Copy)
                            if yd is not None:
                                nc.sync.dma_start(
                                    out=yd[ht, :, t0:t0 + nt, :],
                                    in_=stg.rearrange("p (t b) -> p t b", b=B))
                            else:  # readout: y3 is [O, T3, B]
                                nc.sync.dma_start(
                                    out=y3[m0:m0 + mtw, t0:t0 + nt, :],
                                    in_=stg[:mtw].rearrange(
                                        "p (t b) -> p t b", b=B))

        if c.max_phase >= 4:
            conv_sp(s1d, w1t, H, tts2, yd=y2d, sumt=sum2, sqt=sq2,
                    nslots=n2slots, tag="c2")
        if c.max_phase >= 5:
            bn_affine(sum2, sq2, n2slots, c.T2 * c.B_tot, gam1, bet1,
                      cc2i, cc2o, A2, C2b, "2")
        if c.max_phase >= 6:
            lif_layer(y2d, s2d, A2, C2b, c.T2, "2")
        if c.max_phase >= 7:
            conv_sp(s2d, wrt, O, tts3, y3=y3d, tag="c3")

        # =============== tail: LI scan, softmax over O, sum over t =========
        if c.max_phase < 8:
            with ExitStack() as pt:
                tp0 = pt.enter_context(tc.tile_pool(name="tail0", bufs=1))
                z = tp0.tile([B, O], F32, name="z")
                nc.vector.memset(z, 0.0)
                nc.sync.dma_start(out=out.ap(), in_=z)
        if c.max_phase >= 8:
            with ExitStack() as pt:
              psum = pt.enter_context(tc.tile_pool(name="psumt", bufs=1,
                                                   space="PSUM"))
              tp = pt.enter_context(tc.tile_pool(name="tail", bufs=1))
              tp2 = pt.enter_context(tc.tile_pool(name="tail2", bufs=3))
              TB = c.T3 * B
              Y3 = tp.tile([O, TB], F32, name="Y3")
              nc.sync.dma_start(out=Y3.rearrange("p (t b) -> p t b", b=B),
                                in_=y3d)
              beta_t = tp.tile([128, c.T3], F32, name="beta_t")
              nc.vector.memset(beta_t, float(c.BETA))
              idn = tp.tile([128, 128], F32, name="idn")
              make_identity(nc, idn)
              selbt = tp.tile([128, B], F32, name="selbt")
              nc.sync.dma_start(out=selbt, in_=selb.ap())
              us = tp.tile([O, TB], F32, name="us")
              # LI scan over t, one strided scan per batch column
              usv = us.rearrange("p (t b) -> p b t", b=B)
              y3v = Y3.rearrange("p (t b) -> p b t", b=B)
              for b in range(B):
                  nc.vector.tensor_tensor_scan(
                      out=usv[:, b, :], data0=beta_t[:O], data1=y3v[:, b, :],
                      initial=0.0, op0=OP.mult, op1=OP.add)
              # per-128-col blocks: transpose to (t*b, o), softmax over o, then
              # sum over t via selector matmul into (B, O)
              acc = psum.tile([B, O], F32, tag="accps", name="accps", bufs=1)
              blocks = split_tiles(TB, 128)
              for bi, (c0, cw) in enumerate(blocks):
                  pst = psum.tile([128, O], F32, tag="tpps", name="tpps", bufs=2)
                  nc.tensor.transpose(out=pst[:cw, :O],
                                      in_=us[:, c0:c0 + cw],
                                      identity=idn[:O, :O])
                  v = tp2.tile([128, O], F32, tag="v", name="v")
                  nc.scalar.copy(out=v[:cw], in_=pst[:cw, :O])
                  mx = tp2.tile([128, 1], F32, tag="mx", name="mx")
                  nc.vector.reduce_max(out=mx[:cw], in_=v[:cw],
                                       axis=mybir.AxisListType.X)
                  ev = tp2.tile([128, O], F32, tag="ev", name="ev")
                  nc.vector.tensor_scalar(ev[:cw], v[:cw], mx[:cw], None,
                                          OP.subtract)
                  pv = tp2.tile([128, O], F32, tag="pv", name="pv")
                  sm = tp2.tile([128, 1], F32, tag="sm", name="sm")
                  nc.scalar.activation(out=pv[:cw], in_=ev[:cw], func=AF.Exp,
                                       accum_out=sm[:cw])
                  rsm = tp2.tile([128, 1], F32, tag="rsm", name="rsm")
                  nc.vector.reciprocal(rsm[:cw], sm[:cw])
                  pn_t = tp2.tile([128, O], F32, tag="pnt", name="pnt")
                  nc.vector.tensor_scalar(pn_t[:cw], pv[:cw], rsm[:cw], None,
                                          OP.mult)
                  nc.tensor.matmul(
                      acc, lhsT=selbt[:cw], rhs=pn_t[:cw],
                      start=(bi == 0), stop=(bi == len(blocks) - 1),
                      skip_group_check=True)
              res = tp.tile([B, O], F32, name="res")
              nc.scalar.copy(out=res, in_=acc)
              nc.sync.dma_start(out=out.ap(), in_=res)
        if c.dbg:
            nc.sync.dma_start(out=d_y1.ap(), in_=y1d)
            nc.sync.dma_start(out=d_s1.ap(), in_=s1d)
            nc.sync.dma_start(out=d_y2.ap(), in_=y2d)
            nc.sync.dma_start(out=d_y3.ap(), in_=y3d)

    nc.compile()
    return nc


# ======================= host side =======================

def dcls_np(w, p, K, SIG):
    w = np.asarray(w, np.float32)
    p = np.asarray(p, np.float32)
    idx = np.arange(K, dtype=np.float32)
    d = idx[None, None, :] - np.float32(K // 2) - p[:, :, None]
    t = d / np.float32(SIG)
    g = np.exp(np.float32(-0.5) * t * t).astype(np.float32)
    g = g / (np.sum(g, axis=-1, keepdims=True, dtype=np.float32)
             + np.float32(1e-7))
    return (w[:, :, None] * g).astype(np.float32)


def _np_dt(c):
    return mybir.dt.np(c.MMDT)


def _shared_inputs(cfg: Cfg, w0, p0, g0, b0, w1, p1, g1, b1, wr, pr):
    c = cfg

    def chanmat(v):
        return np.ascontiguousarray(
            np.asarray(v, np.float32).reshape(c.HT, 128).T)

    def T(a):
        return np.ascontiguousarray(np.asarray(a, np.float32).T)

    nrep = max(1, (128 + c.B_loc - 1) // c.B_loc)
    selb = np.ascontiguousarray(
        np.tile(np.eye(c.B_loc, dtype=np.float32), (nrep, 1)))[:128]
    return {
        "selb": selb,
        "w0T": T(w0), "p0T": T(p0),
        "w1T": T(w1), "p1T": T(p1),
        "wrT": T(wr), "prT": T(pr),
        "g0m": chanmat(g0), "b0m": chanmat(b0),
        "g1m": chanmat(g1), "b1m": chanmat(b1),
    }


def _x_timemajor(cfg: Cfg, x):
    """(B_tot, T0, J) -> (n_cores, J, T0+PADT, B_loc) zero-padded, wire dtype."""
    c = cfg
    x = np.asarray(x, np.float32)
    xs = x.reshape(c.n_cores, c.B_loc, c.T0, c.J).transpose(0, 3, 2, 1)
    dt = np.uint16 if c.x_u16 else _np_dt(c)
    out = np.zeros((c.n_cores, c.J, c.T0 + c.PADT, c.B_loc), dt)
    if c.x_u16:
        out[:, :, c.LPAD:c.LPAD + c.T0, :] = np.clip(
            np.rint(xs * np.float32(XSCALE)), 0, XSCALE).astype(np.uint16)
    else:
        out[:, :, c.LPAD:c.LPAD + c.T0, :] = xs
    return out


def make_in_maps(cfg: Cfg, x, **params):
    """Per-core input dicts (sim / run_bass_kernel_spmd path)."""
    c = cfg
    shared = _shared_inputs(cfg, **params)
    xs = _x_timemajor(cfg, x)
    in_maps = []
    for ci in range(c.n_cores):
        m = dict(shared)
        m["xp"] = xs[ci]
        in_maps.append(m)
    return in_maps


def make_concat_inputs(cfg: Cfg, x, **params):
    """Axis-0-concatenated global inputs (cached-jit fast path)."""
    c = cfg
    n = c.n_cores
    shared = _shared_inputs(cfg, **params)
    out = {}
    for name, a in shared.items():
        out[name] = np.tile(a, (n,) + (1,) * (a.ndim - 1))
    xs = _x_timemajor(cfg, x)
    out["xp"] = xs.reshape(n * c.J, c.T0 + c.PADT, c.B_loc)
    return out


_CACHE = {}


def _get_nc(cfg: Cfg):
    key = ("nc", cfg.T0, cfg.B_loc, cfg.J, cfg.H, cfg.O, cfg.K, cfg.n_cores,
           cfg.mm_bf16)
    if key not in _CACHE:
        _CACHE[key] = build_kernel(cfg)
    return _CACHE[key]


class _Runner:
    """Cached PJRT executor: jit(shard_map(bass_exec)) built once, reused
    across kernel() calls. Mirrors bass2jax.run_bass_via_pjrt."""

    def __init__(self, cfg: Cfg):
        import jax
        from jax.sharding import Mesh, PartitionSpec
        try:
            from jax.experimental.shard_map import shard_map
        except ImportError:
            from jax.shard_map import shard_map
        from concourse import bass2jax

        self.cfg = cfg
        self.jax = jax
        nc = _get_nc(cfg)
        bass2jax.install_neuronx_cc_hook()
        partition_name = (nc.partition_id_tensor.name
                          if nc.partition_id_tensor else None)
        in_names, out_names, out_avals, zero_shapes = [], [], [], []
        for alloc in nc.m.functions[0].allocations:
            if not isinstance(alloc, mybir.MemoryLocationSet):
                continue
            name = alloc.memorylocations[0].name
            if alloc.kind == "ExternalInput":
                if name != partition_name:
                    in_names.append(name)
            elif alloc.kind == "ExternalOutput":
                out_names.append(name)
                shape = tuple(alloc.tensor_shape)
                dtype = mybir.dt.np(alloc.dtype)
                out_avals.append(jax.core.ShapedArray(shape, dtype))
                zero_shapes.append((shape, dtype))
        n_params = len(in_names)
        all_names = in_names + out_names + (
            [partition_name] if partition_name else [])
        donate = tuple(range(n_params, n_params + len(out_names)))
        self.in_names = in_names
        self.out_names = out_names
        self.zero_shapes = zero_shapes

        def _body(*args):
            operands = list(args)
            if partition_name is not None:
                operands.append(bass2jax.partition_id_tensor())
            outs = bass2jax._bass_exec_p.bind(
                *operands, out_avals=tuple(out_avals),
                in_names=tuple(all_names), out_names=tuple(out_names),
                lowering_input_output_aliases=(), sim_require_finite=True,
                sim_require_nnan=True, nc=nc)
            return tuple(outs)

        devices = jax.devices()[:cfg.n_cores]
        assert len(devices) == cfg.n_cores
        mesh = Mesh(np.asarray(devices), ("core",))
        in_specs = (PartitionSpec("core"),) * (n_params + len(out_names))
        out_specs = (PartitionSpec("core"),) * len(out_names)
        self.fn = jax.jit(
            shard_map(_body, mesh=mesh, in_specs=in_specs,
                      out_specs=out_specs, check_rep=False),
            donate_argnums=donate, keep_unused=True)

    def __call__(self, concat_inputs):
        n = self.cfg.n_cores
        args = [concat_inputs[name] for name in self.in_names]
        args += [np.zeros((n * s[0], *s[1:]), dt)
                 for (s, dt) in self.zero_shapes]
        outs = self.fn(*args)
        return np.asarray(outs[self.out_names.index("out")])


def _get_runner(cfg: Cfg) -> _Runner:
    key = ("runner", cfg.T0, cfg.B_loc, cfg.J, cfg.H, cfg.O, cfg.K,
           cfg.n_cores, cfg.mm_bf16)
    if key not in _CACHE:
        _CACHE[key] = _Runner(cfg)
    return _CACHE[key]


def run(cfg: Cfg, inputs, trace=False):
    """Reference path through run_bass_kernel_spmd (uncached jit)."""
    nc = _get_nc(cfg)
    in_maps = make_in_maps(cfg, **inputs)
    res = run_bass_kernel_spmd(nc, in_maps, core_ids=list(range(cfg.n_cores)),
                               trace=trace)
    outs = [res.results[ci]["out"].reshape(cfg.B_loc, cfg.O)
            for ci in range(cfg.n_cores)]
    return np.concatenate(outs, axis=0), res


def run_fast(cfg: Cfg, inputs):
    r = _get_runner(cfg)
    ci = make_concat_inputs(cfg, **inputs)
    out = r(ci)
    return out.reshape(cfg.B_tot, cfg.O)


def kernel(**inputs):
    cfg = Cfg()
    return run_fast(cfg, inputs)


# revision 27
# speedup vs baseline: 25.3542x; 1.5840x over previous
"""Trainium2 Bass kernel for nn_DelayLIFSNN.

Architecture (per reference):
  x (B, T0, J) -> delay_conv(w0,p0) -> BN(global batch stats) -> LIF
               -> delay_conv(w1,p1) -> BN -> LIF
               -> delay_conv(wr,pr) -> LI readout -> sum_t softmax_o -> (B, O)

Sharding: data-parallel over batch B across 8 cores (B_loc=32/core);
weights replicated; BN stats all-reduced ((128, 2*HT) f32 = 4KB each).

Host->device traffic is the bottleneck (axon tunnel ~60 MB/s), so the
Dcls gaussian delay kernels are expanded ON DEVICE from the raw (w, p)
parameters (2.7 MB/core) instead of shipping the K=25-expanded conv
kernels (34 MB/core). The jitted PJRT executable is cached across
kernel() calls.

Conv = sum over K=25 taps of shifted matmuls accumulated in PSUM.
LIF = per-step scalar_tensor_tensor ops on DVE (sequential over time).
LI readout = tensor_tensor_scan. Softmax+time-sum via PE transpose + ones-matmul.

Activation layouts:
  x / spikes (conv rhs): [ch_tile][ch_part 128, t*B + b]   (DRAM: [CT,128,Tpad,B])
  conv out psum:         [out_part 128, t*B + b] per (ht, time-tile)
  y DRAM:                [HT, 128, T, B]
  LIF scan tiles:        [h_part 128, t*(HT*B) + ht*B + b]
  readout y3 DRAM:       [B*O, T3]  (rows b*O+o)
"""

import sys
import numpy as np

try:
    import concourse.bass as bass
except ImportError:  # grading env fallback
    sys.path.insert(0, "/opt/trn_rl_repo")
    import concourse.bass as bass

import concourse.mybir as mybir
import concourse.tile as tile
from contextlib import ExitStack
from concourse import bacc
from concourse.bass_utils import run_bass_kernel_spmd
from concourse.masks import make_identity

F32 = mybir.dt.float32
BF16 = mybir.dt.bfloat16
U16 = mybir.dt.uint16
AF = mybir.ActivationFunctionType
OP = mybir.AluOpType
XSCALE = 65535.0  # u16 fixed-point scale for x in [0, 1]


class Cfg:
    def __init__(self, T0=300, B_loc=32, J=140, H=512, O=20, K=25, n_cores=8,
                 BETA=0.95, THRESH=1.0, SIG=0.5, EPS=1e-5, NT=16, CH=48,
                 CHUNK_TT=6, mm_bf16=False, x_u16=True, dbg=False,
                 max_phase=9, ablate=()):
        self.T0, self.B_loc, self.J, self.H, self.O, self.K = T0, B_loc, J, H, O, K
        self.n_cores = n_cores
        self.BETA, self.THRESH, self.SIG, self.EPS = BETA, THRESH, SIG, EPS
        self.LPAD, self.RPAD = K - 1, (K - 1) // 2
        self.PADT = self.LPAD + self.RPAD                      # 36
        self.T1 = T0 + self.RPAD                               # 312
        self.T2 = self.T1 + self.RPAD                          # 324
        self.T3 = self.T2 + self.RPAD                          # 336
        self.NT = NT                                           # out-steps per matmul tile
        self.CH = CH                                           # LIF chunk steps
        self.CHUNK_TT = CHUNK_TT                               # time-tiles per psum chunk
        self.HT = (H + 127) // 128                             # h tiles (4)
        self.B_tot = B_loc * n_cores
        self.mm_bf16 = mm_bf16                                 # matmul operand dtype
        self.MMDT = BF16 if mm_bf16 else F32
        self.x_u16 = x_u16                                     # ship x as u16 fixed point
        self.XDT = U16 if x_u16 else self.MMDT
        self.dbg = dbg
        self.max_phase = max_phase
        self.ablate = set(ablate)
        self.J0 = min(J, 128)
        self.JL = J - self.J0                                  # leftover channels (12)


def split_tiles(total, size):
    out = []
    t = 0
    while t < total:
        n = min(size, total - t)
        out.append((t, n))
        t += n
    return out


def bc(ap, axis, count):
    """Insert a stride-0 (broadcast) axis at position `axis` of an AP."""
    dims = [list(d) for d in ap.ap]
    dims.insert(axis, [0, count])
    return bass.AP(tensor=ap.tensor, offset=ap.offset, ap=dims)


def param_rows(c: Cfg):
    """Row offsets of each packed param in the [R, H] param pack."""
    PC = c.H
    r = {}
    pos = 0
    for name, n in (("w0T", c.J * c.H), ("p0T", c.J * c.H),
                    ("w1T", c.H * c.H), ("p1T", c.H * c.H),
                    ("wrT", c.H * c.O), ("prT", c.H * c.O),
                    ("g0m", 128 * c.HT), ("b0m", 128 * c.HT),
                    ("g1m", 128 * c.HT), ("b1m", 128 * c.HT),
                    ("selb", 128 * c.B_loc)):
        assert n % PC == 0
        r[name] = pos
        pos += n // PC
    r["tot"] = pos
    r["pad"] = ((pos + c.n_cores - 1) // c.n_cores) * c.n_cores
    return r


def build_kernel(cfg: Cfg):
    c = cfg
    B, HT, K, H, O = c.B_loc, c.HT, c.K, c.H, c.O
    MMDT = c.MMDT
    nc = bacc.Bacc("TRN2", target_bir_lowering=False, debug=False,
                   num_devices=c.n_cores)

    tts1 = split_tiles(c.T1, c.NT)
    tts2 = split_tiles(c.T2, c.NT)
    tts3 = split_tiles(c.T3, c.NT)
    n1slots = len(tts1)
    n2slots = len(tts2)

    # ---- I/O ----
    # x, time-major transposed AND zero-padded on host (u16 zeros = 0.0)
    xp = nc.dram_tensor("xp", [c.J, c.T0 + c.PADT, B], c.XDT,
                        kind="ExternalInput")
    # All replicated params ship as ONE per-core 1/8 slice of a packed
    # [R, PC] f32 buffer; an on-device AllGather reconstructs the full
    # pack on every core (saves 8x on the host->device tunnel).
    PC = H  # pack row width; all param sizes are multiples of H=512
    rows = param_rows(c)
    ppk = nc.dram_tensor("ppk", [rows["pad"] // c.n_cores, PC], F32,
                         kind="ExternalInput")
    out = nc.dram_tensor("out", [B, O], F32, kind="ExternalOutput")
    if c.dbg:
        d_y1 = nc.dram_tensor("d_y1", [HT, 128, c.T1, B], F32, kind="ExternalOutput")
        d_s1 = nc.dram_tensor("d_s1", [HT, 128, c.T1 + c.PADT, B], F32, kind="ExternalOutput")
        d_y2 = nc.dram_tensor("d_y2", [HT, 128, c.T2, B], F32, kind="ExternalOutput")
        d_y3 = nc.dram_tensor("d_y3", [O, c.T3, B], F32, kind="ExternalOutput")
        d_w = nc.dram_tensor("d_w", [K, c.J0, H], F32, kind="ExternalOutput")

    with tile.TileContext(nc) as tc, ExitStack() as ctx:
        dram = ctx.enter_context(tc.tile_pool(name="dram", bufs=1, space="DRAM"))
        # expanded Dcls kernels, produced on device
        w0a = dram.tile([K, c.J0, H], MMDT, name="w0a")
        w0b = dram.tile([K, c.JL, H], MMDT, name="w0b") if c.JL else None
        w1t = dram.tile([K, H, H], MMDT, name="w1t")
        wrt = dram.tile([K, H, O], MMDT, name="wrt")
        y1d = dram.tile([HT, 128, c.T1, B], F32, name="y1d")
        s1d = dram.tile([HT, 128, c.T1 + c.PADT, B], MMDT, name="s1d")
        y2d = dram.tile([HT, 128, c.T2, B], F32, name="y2d")
        s2d = dram.tile([HT, 128, c.T2 + c.PADT, B], MMDT, name="s2d")
        y3d = dram.tile([O, c.T3, B], F32, name="y3d")
        cc_space = "Shared" if c.n_cores > 4 else "Local"
        cc1i = dram.tile([128, 2 * HT], F32, name="cc1i")
        cc1o = dram.tile([128, 2 * HT], F32, name="cc1o", addr_space=cc_space)
        cc2i = dram.tile([128, 2 * HT], F32, name="cc2i")
        cc2o = dram.tile([128, 2 * HT], F32, name="cc2o", addr_space=cc_space)

        # gather the full param pack from the per-core 1/8 slices
        # (collectives can't touch IO tensors; bounce input via internal DRAM)
        ppi = dram.tile([rows["pad"] // c.n_cores, PC], F32, name="ppi")
        nc.sync.dma_start(out=ppi, in_=ppk.ap())
        ppg = dram.tile([rows["pad"], PC], F32, name="ppg",
                        addr_space=cc_space)
        nc.gpsimd.collective_compute(
            "AllGather", OP.bypass,
            replica_groups=[list(range(c.n_cores))],
            ins=[ppi], outs=[ppg])

        def pview(name, shape):
            """AP view of packed param `name` with 2-D `shape`."""
            nrows = (shape[0] * shape[1]) // PC
            v = ppg[rows[name]:rows[name] + nrows, :]
            if shape[1] == PC:
                return v
            return (v.rearrange("a b -> (a b)")
                     .rearrange("(i j) -> i j", j=shape[1]))

        glob = ctx.enter_context(tc.tile_pool(name="glob", bufs=1))

        # persistent small tiles
        sum1 = glob.tile([128, HT * n1slots], F32, name="sum1")
        sq1 = glob.tile([128, HT * n1slots], F32, name="sq1")
        sum2 = glob.tile([128, HT * n2slots], F32, name="sum2")
        sq2 = glob.tile([128, HT * n2slots], F32, name="sq2")
        gam0 = glob.tile([128, HT], F32, name="gam0")
        bet0 = glob.tile([128, HT], F32, name="bet0")
        gam1 = glob.tile([128, HT], F32, name="gam1")
        bet1 = glob.tile([128, HT], F32, name="bet1")
        if "nogb" not in c.ablate:
            nc.sync.dma_start(out=gam0, in_=pview("g0m", (128, HT)))
            nc.sync.dma_start(out=bet0, in_=pview("b0m", (128, HT)))
            nc.sync.dma_start(out=gam1, in_=pview("g1m", (128, HT)))
            nc.sync.dma_start(out=bet1, in_=pview("b1m", (128, HT)))
        A1 = glob.tile([128, HT], F32, name="A1")
        C1b = glob.tile([128, HT * B], F32, name="C1b")
        A2 = glob.tile([128, HT], F32, name="A2")
        C2b = glob.tile([128, HT * B], F32, name="C2b")
        zpad = glob.tile([128, c.LPAD * B], MMDT, name="zpad")
        nc.vector.memset(zpad, 0.0)

        # zero the pad regions of the spike dram buffers
        for sd, T in (() if "nozpad" in c.ablate else ((s1d, c.T1), (s2d, c.T2))):
            for ht in range(HT):
                nc.sync.dma_start(out=sd[ht, :, 0:c.LPAD, :],
                                  in_=zpad.rearrange("p (t b) -> p t b", b=B))
                nc.sync.dma_start(
                    out=sd[ht, :, T + c.LPAD:T + c.PADT, :],
                    in_=zpad.rearrange("p (t b) -> p t b", b=B)[:, :c.RPAD, :])

        # =============== Phase 0: expand Dcls kernels on device ===========
        # g_k = exp(-0.5*((k - K//2 - p)/SIG)^2), normalized over k, * w.
        # SIG=0.5 -> exp(-2*(p - (k - K//2))^2).
        def expand(wsrc, psrc, r0, rp, M, dst, dstsl, tag):
            """rows r0:r0+rp of wsrc/psrc ([R, M] dram) -> dst[k][dstsl, :M]."""
            with ExitStack() as pe:
                ep = pe.enter_context(tc.tile_pool(name=f"exp{tag}", bufs=1))
                wp = ep.tile([rp, M], F32, name=f"wp{tag}")
                pp = ep.tile([rp, M], F32, name=f"pp{tag}")
                nc.sync.dma_start(out=wp, in_=wsrc[r0:r0 + rp, :])
                nc.sync.dma_start(out=pp, in_=psrc[r0:r0 + rp, :])
                eb = ep.tile([rp, K * M], F32, name=f"eb{tag}")
                ss = ep.tile([rp, M], F32, name=f"ss{tag}")
                dk = ep.tile([rp, M], F32, name=f"dk{tag}")
                sq = ep.tile([rp, M], F32, name=f"sq{tag}")
                inv2s2 = -1.0 / (2.0 * c.SIG * c.SIG)
                for k in range(K):
                    ek = eb[:, k * M:(k + 1) * M]
                    nc.vector.tensor_scalar(dk, pp, float(k - K // 2), None,
                                            OP.subtract)
                    nc.scalar.activation(out=sq, in_=dk, func=AF.Square)
                    nc.scalar.activation(out=ek, in_=sq, func=AF.Exp,
                                         scale=float(inv2s2))
                    if k == 0:
                        nc.vector.tensor_copy(ss, ek)
                    else:
                        nc.vector.tensor_add(ss, ss, ek)
                nc.vector.tensor_scalar_add(ss, ss, 1e-7)
                rn = ep.tile([rp, M], F32, name=f"rn{tag}")
                nc.vector.reciprocal(rn, ss)
                wn = ep.tile([rp, M], F32, name=f"wn{tag}")
                nc.vector.tensor_mul(wn, wp, rn)
                ok = ep.tile([rp, K * M], MMDT, name=f"ok{tag}")
                for k in range(K):
                    nc.vector.tensor_mul(ok[:, k * M:(k + 1) * M],
                                         eb[:, k * M:(k + 1) * M], wn)
                for k in range(K):
                    nc.sync.dma_start(out=dst[k, dstsl, :],
                                      in_=ok[:, k * M:(k + 1) * M])

        w0v, p0v = pview("w0T", (c.J, H)), pview("p0T", (c.J, H))
        w1v, p1v = pview("w1T", (H, H)), pview("p1T", (H, H))
        wrv, prv = pview("wrT", (H, O)), pview("prT", (H, O))
        expand(w0v, p0v, 0, c.J0, H, w0a, slice(0, c.J0), "0a")
        if c.JL:
            expand(w0v, p0v, c.J0, c.JL, H, w0b, slice(0, c.JL), "0b")
        for ct in range(HT):
            expand(w1v, p1v, ct * 128, 128, H, w1t,
                   slice(ct * 128, (ct + 1) * 128), f"1_{ct}")
            expand(wrv, prv, ct * 128, 128, O, wrt,
                   slice(ct * 128, (ct + 1) * 128), f"r_{ct}")
        if c.dbg:
            nc.sync.dma_start(out=d_w.ap(), in_=w0a)

        # =============== Phase 1: conv1 (x -> y1) + stats ===============
        with ExitStack() as p1:
            psum = p1.enter_context(tc.tile_pool(name="psum1", bufs=8,
                                                  space="PSUM"))
            xpool = p1.enter_context(tc.tile_pool(name="xpool", bufs=1))

            T0p = c.T0 + c.PADT

            with ExitStack() as pxu:
                xup = (pxu.enter_context(tc.tile_pool(name="xup", bufs=1))
                       if c.x_u16 else None)

                def load_x(jp, j0):
                    """[jp, T0p*B] MMDT tile (host pre-padded), dequant."""
                    X = xpool.tile([jp, T0p * B], MMDT, name=f"X_{j0}")
                    if c.x_u16:
                        XU = xup.tile([jp, T0p * B], U16, name=f"XU_{j0}")
                        nc.sync.dma_start(
                            out=XU.rearrange("p (t b) -> p t b", b=B),
                            in_=xp.ap()[j0:j0 + jp])
                        nc.scalar.activation(out=X, in_=XU, func=AF.Copy,
                                             scale=float(1.0 / XSCALE))
                    else:
                        nc.sync.dma_start(
                            out=X.rearrange("p (t b) -> p t b", b=B),
                            in_=xp.ap()[j0:j0 + jp])
                    return X

                X0 = load_x(c.J0, 0)
                X1 = load_x(c.JL, c.J0) if c.JL else None

            wpool1 = p1.enter_context(tc.tile_pool(name="wpool1", bufs=1))
            stg1 = p1.enter_context(tc.tile_pool(name="stg1", bufs=3))
            W0 = wpool1.tile([c.J0, K * H], MMDT, name="W0")
            nc.sync.dma_start(out=W0.rearrange("p (k h) -> p k h", h=H),
                              in_=w0a.rearrange("k p h -> p k h"))
            if c.JL:
                W1l = wpool1.tile([c.JL, K * H], MMDT, name="W1l")
                nc.sync.dma_start(out=W1l.rearrange("p (k h) -> p k h", h=H),
                                  in_=w0b.rearrange("k p h -> p k h"))

            n_mm = K * (2 if c.JL else 1)
            zsrc = None
            if "nomm" in c.ablate:
                zsrc = wpool1.tile([128, c.NT * B], F32, name="zsrc")
                nc.vector.memset(zsrc, 0.0)
            for tti, (t0, nt) in enumerate(tts1):
                for ht in range(HT):
                    if "nomm" not in c.ablate:
                        ps = psum.tile([128, nt * B], F32, tag="cv1ps", name="ps1")
                        mi = 0
                        for kk in range(K):
                            nc.tensor.matmul(
                                ps, lhsT=W0[:, kk * H + ht * 128: kk * H + ht * 128 + 128],
                                rhs=X0[:, (t0 + kk) * B:(t0 + kk) * B + nt * B],
                                start=(mi == 0), stop=(mi == n_mm - 1))
                            mi += 1
                            if c.JL:
                                nc.tensor.matmul(
                                    ps,
                                    lhsT=W1l[:, kk * H + ht * 128: kk * H + ht * 128 + 128],
                                    rhs=X1[:, (t0 + kk) * B:(t0 + kk) * B + nt * B],
                                    start=(mi == 0), stop=(mi == n_mm - 1))
                                mi += 1
                        src = ps
                    else:
                        src = zsrc[:, :nt * B]
                    slot = ht * n1slots + tti
                    ystg = stg1.tile([128, nt * B], F32, tag="ystg", name="ystg")
                    if "nostats" not in c.ablate:
                        nc.scalar.activation(out=ystg, in_=src, func=AF.Copy,
                                             accum_out=sum1[:, slot:slot + 1])
                        ysq = stg1.tile([128, nt * B], F32, tag="ysq", name="ysq")
                        nc.scalar.activation(out=ysq, in_=src, func=AF.Square,
                                             accum_out=sq1[:, slot:slot + 1])
                    else:
                        nc.scalar.activation(out=ystg, in_=src, func=AF.Copy)
                    if "nostore" not in c.ablate:
                        nc.sync.dma_start(
                            out=y1d[ht, :, t0:t0 + nt, :],
                            in_=ystg.rearrange("p (t b) -> p t b", b=B))

        # =============== BN stats: allreduce + affine ===============
        def bn_affine(sumt, sqt, nslots, N, gam, bet, cci, cco, A, Cb, tagp):
            with ExitStack() as pb:
                sp = pb.enter_context(tc.tile_pool(name=f"bn{tagp}", bufs=1))
                ccs = sp.tile([128, 2 * HT], F32, name=f"ccs{tagp}")
                nc.vector.reduce_sum(
                    out=ccs[:, 0:HT],
                    in_=sumt.rearrange("p (h s) -> p h s", s=nslots),
                    axis=mybir.AxisListType.X)
                nc.vector.reduce_sum(
                    out=ccs[:, HT:2 * HT],
                    in_=sqt.rearrange("p (h s) -> p h s", s=nslots),
                    axis=mybir.AxisListType.X)
                nc.sync.dma_start(out=cci, in_=ccs)
                nc.gpsimd.collective_compute(
                    "AllReduce", OP.add,
                    replica_groups=[list(range(c.n_cores))],
                    ins=[cci], outs=[cco])
                gs = sp.tile([128, 2 * HT], F32, name=f"gs{tagp}")
                nc.sync.dma_start(out=gs, in_=cco)
                rN = float(1.0 / N)
                mu = sp.tile([128, HT], F32, name=f"mu{tagp}")
                nc.vector.tensor_scalar(mu, gs[:, 0:HT], rN, None, OP.mult)
                ex2 = sp.tile([128, HT], F32, name=f"ex2{tagp}")
                nc.vector.tensor_scalar(ex2, gs[:, HT:2 * HT], rN, None,
                                        OP.mult)
                var = sp.tile([128, HT], F32, name=f"var{tagp}")
                # var = ex2 - mu*mu ; then + eps
                nc.vector.scalar_tensor_tensor(out=var, in0=mu, scalar=1.0,
                                               in1=mu, op0=OP.mult, op1=OP.mult)
                nc.vector.tensor_sub(var, ex2, var)
                nc.vector.tensor_scalar_add(var, var, float(c.EPS))
                sv = sp.tile([128, HT], F32, name=f"sv{tagp}")
                nc.scalar.activation(out=sv, in_=var, func=AF.Sqrt)
                # one Newton step: s' = 0.5*(s + v/s)  (ACT sqrt is ~3e-6 approx)
                rs0 = sp.tile([128, HT], F32, name=f"rs0{tagp}")
                nc.vector.reciprocal(rs0, sv)
                t1 = sp.tile([128, HT], F32, name=f"t1{tagp}")
                nc.vector.tensor_mul(t1, var, rs0)
                nc.vector.tensor_add(sv, sv, t1)
                nc.vector.tensor_scalar(sv, sv, 0.5, None, OP.mult)
                rsv = sp.tile([128, HT], F32, name=f"rsv{tagp}")
                nc.vector.reciprocal(rsv, sv)
                nc.vector.tensor_mul(A, gam, rsv)
                # Cbias = bet - mu*A, broadcast over batch
                cb1 = sp.tile([128, HT], F32, name=f"cb1{tagp}")
                nc.vector.tensor_mul(cb1, mu, A)
                nc.vector.tensor_sub(cb1, bet, cb1)
                nc.vector.tensor_copy(
                    Cb.rearrange("p (h b) -> p h b", b=B), bc(cb1, 2, B))

        if c.max_phase >= 2:
            bn_affine(sum1, sq1, n1slots, c.T1 * c.B_tot, gam0, bet0,
                      cc1i, cc1o, A1, C1b, "1")

        # =============== LIF layer (generic) ===============
        def lif_layer(yd, sd, A, Cb, T, tag):
            with ExitStack() as pl:
                lp = pl.enter_context(tc.tile_pool(name=f"lif{tag}", bufs=2))
                up = pl.enter_context(tc.tile_pool(name=f"lifu{tag}", bufs=1))
                HTB = HT * B
                U = up.tile([128, HTB], F32, name=f"U{tag}")
                nc.vector.memset(U, 0.0)
                for (c0, cn) in split_tiles(T, c.CH):
                    ybufs = []
                    for ht in range(HT):
                        yb = lp.tile([128, cn * B], F32, tag=f"yb{ht}",
                                     name=f"yb{tag}")
                        nc.sync.dma_start(
                            out=yb.rearrange("p (t b) -> p t b", b=B),
                            in_=yd[ht, :, c0:c0 + cn, :])
                        ybufs.append(yb)
                    scn = lp.tile([128, cn * HTB], F32, tag="scn",
                                  name=f"scn{tag}")
                    scn3 = scn.rearrange("p (t x) -> p t x", x=HTB)
                    for ht in range(HT):
                        nc.vector.scalar_tensor_tensor(
                            out=scn3[:, :, ht * B:(ht + 1) * B],
                            in0=ybufs[ht].rearrange("p (t b) -> p t b", b=B),
                            scalar=A[:, ht:ht + 1],
                            in1=bc(Cb[:, ht * B:(ht + 1) * B], 1, cn),
                            op0=OP.mult, op1=OP.add)
                    S = lp.tile([128, cn * HTB], MMDT, tag="S", name=f"S{tag}")
                    for t in range(cn):
                        sl = slice(t * HTB, (t + 1) * HTB)
                        ut = lp.tile([128, HTB], F32, tag="ut", name=f"ut{tag}")
                        nc.vector.scalar_tensor_tensor(
                            out=ut, in0=U, scalar=float(c.BETA),
                            in1=scn[:, sl], op0=OP.mult, op1=OP.add)
                        nc.vector.tensor_scalar(
                            S[:, sl], ut, float(c.THRESH), None, OP.is_ge)
                        nc.vector.scalar_tensor_tensor(
                            out=U, in0=ut, scalar=float(c.THRESH), in1=ut,
                            op0=OP.is_lt, op1=OP.mult)
                    S3 = S.rearrange("p (t h b) -> p t h b", h=HT, b=B)
                    for ht in range(HT):
                        nc.sync.dma_start(
                            out=sd[ht, :, c.LPAD + c0:c.LPAD + c0 + cn, :],
                            in_=S3[:, :, ht, :])

        if c.max_phase >= 3:
            lif_layer(y1d, s1d, A1, C1b, c.T1, "1")

        # =============== conv from spikes (generic: layer 2 & readout) =====
        def conv_sp(sd, wsrc, M, tts, yd=None, sumt=None, sqt=None,
                    nslots=0, y3=None, tag=""):
            """y[o, t] = sum_{ct,k} W_k[ct]^T s[ct, t+k] (padded s)."""
            MT = (M + 127) // 128
            tchunks = split_tiles(len(tts), c.CHUNK_TT)
            with ExitStack() as pc:
                psum = pc.enter_context(tc.tile_pool(name=f"psum{tag}",
                                                     bufs=8, space="PSUM"))
                swp = pc.enter_context(tc.tile_pool(name=f"swin{tag}", bufs=2))
                wp = pc.enter_context(tc.tile_pool(name=f"w{tag}", bufs=3))
                sg = pc.enter_context(tc.tile_pool(name=f"stg{tag}", bufs=3))
                for (tci, ntt) in tchunks:
                    tt_group = tts[tci:tci + ntt]
                    w0_ = tt_group[0][0]
                    last_t0, last_nt = tt_group[-1]
                    winlen = (last_t0 + last_nt - 1 + K - 1) - w0_ + 1
                    swin = []
                    for ct in range(HT):
                        sw = swp.tile([128, winlen * B], MMDT, tag=f"sw{ct}",
                                      name=f"sw{tag}")
                        nc.sync.dma_start(
                            out=sw.rearrange("p (t b) -> p t b", b=B),
                            in_=sd[ct, :, w0_:w0_ + winlen, :])
                        swin.append(sw)
                    for ht in range(MT):
                        m0 = ht * 128
                        mtw = min(128, M - m0)
                        pss = [psum.tile([128, nt * B], F32, tag="cvps",
                                         name=f"ps{tag}")
                               for (t0, nt) in tt_group]
                        n_acc = HT * K
                        mi = 0
                        for ct in range(HT):
                            wt = wp.tile([128, K * mtw], MMDT, tag="wt",
                                         name=f"wt{tag}")
                            nc.sync.dma_start(
                                out=wt.rearrange("p (k m) -> p k m", m=mtw),
                                in_=wsrc[:, ct * 128:(ct + 1) * 128,
                                         m0:m0 + mtw].rearrange(
                                             "k p m -> p k m"))
                            for kk in range(K):
                                lhsT = wt[:, kk * mtw:(kk + 1) * mtw]
                                st = (mi == 0)
                                sp_ = (mi == n_acc - 1)
                                for ti, (t0, nt) in enumerate(tt_group):
                                    off = (t0 - w0_ + kk) * B
                                    nc.tensor.matmul(
                                        pss[ti][:mtw], lhsT=lhsT,
                                        rhs=swin[ct][:, off:off + nt * B],
                                        start=st, stop=sp_)
                                mi += 1
                        for ti, (t0, nt) in enumerate(tt_group):
                            stg = sg.tile([128, nt * B], F32, tag="stg",
                                          name=f"stg{tag}")
                            if sumt is not None:
                                slot = ht * nslots + tci + ti
                                nc.scalar.activation(
                                    out=stg[:mtw], in_=pss[ti][:mtw],
                                    func=AF.Copy,
                                    accum_out=sumt[:, slot:slot + 1])
                                sqg = sg.tile([128, nt * B], F32, tag="sqg",
                                              name=f"sqg{tag}")
                                nc.scalar.activation(
                                    out=sqg[:mtw], in_=pss[ti][:mtw],
                                    func=AF.Square,
                                    accum_out=sqt[:, slot:slot + 1])
                            else:
                                nc.scalar.activation(out=stg[:mtw],
                                                     in_=pss[ti][:mtw],
                                                     func=AF.Copy)
                            if yd is not None:
                                nc.sync.dma_start(
                                    out=yd[ht, :, t0:t0 + nt, :],
                                    in_=stg.rearrange("p (t b) -> p t b", b=B))
                            else:  # readout: y3 is [O, T3, B]
                                nc.sync.dma_start(
                                    out=y3[m0:m0 + mtw, t0:t0 + nt, :],
                                    in_=stg[:mtw].rearrange(
                                        "p (t b) -> p t b", b=B))

        if c.max_phase >= 4:
            conv_sp(s1d, w1t, H, tts2, yd=y2d, sumt=sum2, sqt=sq2,
                    nslots=n2slots, tag="c2")
        if c.max_phase >= 5:
            bn_affine(sum2, sq2, n2slots, c.T2 * c.B_tot, gam1, bet1,
                      cc2i, cc2o, A2, C2b, "2")
        if c.max_phase >= 6:
            lif_layer(y2d, s2d, A2, C2b, c.T2, "2")
        if c.max_phase >= 7:
            conv_sp(s2d, wrt, O, tts3, y3=y3d, tag="c3")

        # =============== tail: LI scan, softmax over O, sum over t =========
        if c.max_phase < 8:
            with ExitStack() as pt:
                tp0 = pt.enter_context(tc.tile_pool(name="tail0", bufs=1))
                z = tp0.tile([B, O], F32, name="z")
                nc.vector.memset(z, 0.0)
                nc.sync.dma_start(out=out.ap(), in_=z)
        if c.max_phase >= 8:
            with ExitStack() as pt:
              psum = pt.enter_context(tc.tile_pool(name="psumt", bufs=1,
                                                   space="PSUM"))
              tp = pt.enter_context(tc.tile_pool(name="tail", bufs=1))
              tp2 = pt.enter_context(tc.tile_pool(name="tail2", bufs=3))
              TB = c.T3 * B
              Y3 = tp.tile([O, TB], F32, name="Y3")
              nc.sync.dma_start(out=Y3.rearrange("p (t b) -> p t b", b=B),
                                in_=y3d)
              beta_t = tp.tile([128, c.T3], F32, name="beta_t")
              nc.vector.memset(beta_t, float(c.BETA))
              idn = tp.tile([128, 128], F32, name="idn")
              make_identity(nc, idn)
              selbt = tp.tile([128, B], F32, name="selbt")
              nc.sync.dma_start(out=selbt, in_=pview("selb", (128, B)))
              us = tp.tile([O, TB], F32, name="us")
              # LI scan over t, one strided scan per batch column
              usv = us.rearrange("p (t b) -> p b t", b=B)
              y3v = Y3.rearrange("p (t b) -> p b t", b=B)
              for b in range(B):
                  nc.vector.tensor_tensor_scan(
                      out=usv[:, b, :], data0=beta_t[:O], data1=y3v[:, b, :],
                      initial=0.0, op0=OP.mult, op1=OP.add)
              # per-128-col blocks: transpose to (t*b, o), softmax over o, then
              # sum over t via selector matmul into (B, O)
              acc = psum.tile([B, O], F32, tag="accps", name="accps", bufs=1)
              blocks = split_tiles(TB, 128)
              for bi, (c0, cw) in enumerate(blocks):
                  pst = psum.tile([128, O], F32, tag="tpps", name="tpps", bufs=2)
                  nc.tensor.transpose(out=pst[:cw, :O],
                                      in_=us[:, c0:c0 + cw],
                                      identity=idn[:O, :O])
                  v = tp2.tile([128, O], F32, tag="v", name="v")
                  nc.scalar.copy(out=v[:cw], in_=pst[:cw, :O])
                  mx = tp2.tile([128, 1], F32, tag="mx", name="mx")
                  nc.vector.reduce_max(out=mx[:cw], in_=v[:cw],
                                       axis=mybir.AxisListType.X)
                  ev = tp2.tile([128, O], F32, tag="ev", name="ev")
                  nc.vector.tensor_scalar(ev[:cw], v[:cw], mx[:cw], None,
                                          OP.subtract)
                  pv = tp2.tile([128, O], F32, tag="pv", name="pv")
                  sm = tp2.tile([128, 1], F32, tag="sm", name="sm")
                  nc.scalar.activation(out=pv[:cw], in_=ev[:cw], func=AF.Exp,
                                       accum_out=sm[:cw])
                  rsm = tp2.tile([128, 1], F32, tag="rsm", name="rsm")
                  nc.vector.reciprocal(rsm[:cw], sm[:cw])
                  pn_t = tp2.tile([128, O], F32, tag="pnt", name="pnt")
                  nc.vector.tensor_scalar(pn_t[:cw], pv[:cw], rsm[:cw], None,
                                          OP.mult)
                  nc.tensor.matmul(
                      acc, lhsT=selbt[:cw], rhs=pn_t[:cw],
                      start=(bi == 0), stop=(bi == len(blocks) - 1),
                      skip_group_check=True)
              res = tp.tile([B, O], F32, name="res")
              nc.scalar.copy(out=res, in_=acc)
              nc.sync.dma_start(out=out.ap(), in_=res)
        if c.dbg:
            nc.sync.dma_start(out=d_y1.ap(), in_=y1d)
            nc.sync.dma_start(out=d_s1.ap(), in_=s1d)
            nc.sync.dma_start(out=d_y2.ap(), in_=y2d)
            nc.sync.dma_start(out=d_y3.ap(), in_=y3d)

    nc.compile()
    return nc


# ======================= host side =======================

def dcls_np(w, p, K, SIG):
    w = np.asarray(w, np.float32)
    p = np.asarray(p, np.float32)
    idx = np.arange(K, dtype=np.float32)
    d = idx[None, None, :] - np.float32(K // 2) - p[:, :, None]
    t = d / np.float32(SIG)
    g = np.exp(np.float32(-0.5) * t * t).astype(np.float32)
    g = g / (np.sum(g, axis=-1, keepdims=True, dtype=np.float32)
             + np.float32(1e-7))
    return (w[:, :, None] * g).astype(np.float32)


def _np_dt(c):
    return mybir.dt.np(c.MMDT)


def _param_pack(cfg: Cfg, w0, p0, g0, b0, w1, p1, g1, b1, wr, pr):
    """Packed [R_pad, H] f32 param buffer (order must match param_rows)."""
    c = cfg
    PC = c.H

    def chanmat(v):
        return np.ascontiguousarray(
            np.asarray(v, np.float32).reshape(c.HT, 128).T)

    def T(a):
        return np.ascontiguousarray(np.asarray(a, np.float32).T)

    nrep = max(1, (128 + c.B_loc - 1) // c.B_loc)
    selb = np.ascontiguousarray(
        np.tile(np.eye(c.B_loc, dtype=np.float32), (nrep, 1)))[:128]
    parts = [T(w0), T(p0), T(w1), T(p1), T(wr), T(pr),
             chanmat(g0), chanmat(b0), chanmat(g1), chanmat(b1), selb]
    rows = param_rows(c)
    flat = np.zeros(rows["pad"] * PC, np.float32)
    pos = 0
    for p in parts:
        n = p.size
        flat[pos:pos + n] = np.ascontiguousarray(p, np.float32).reshape(-1)
        pos += n
    assert pos == rows["tot"] * PC
    return flat.reshape(rows["pad"], PC)


def _x_timemajor(cfg: Cfg, x):
    """(B_tot, T0, J) -> (n_cores, J, T0+PADT, B_loc) zero-padded, wire dtype."""
    c = cfg
    x = np.asarray(x, np.float32)
    xs = x.reshape(c.n_cores, c.B_loc, c.T0, c.J).transpose(0, 3, 2, 1)
    dt = np.uint16 if c.x_u16 else _np_dt(c)
    out = np.zeros((c.n_cores, c.J, c.T0 + c.PADT, c.B_loc), dt)
    if c.x_u16:
        out[:, :, c.LPAD:c.LPAD + c.T0, :] = np.clip(
            np.rint(xs * np.float32(XSCALE)), 0, XSCALE).astype(np.uint16)
    else:
        out[:, :, c.LPAD:c.LPAD + c.T0, :] = xs
    return out


def make_in_maps(cfg: Cfg, x, **params):
    """Per-core input dicts (sim / run_bass_kernel_spmd path)."""
    c = cfg
    pack = _param_pack(cfg, **params)
    pr = pack.shape[0] // c.n_cores
    xs = _x_timemajor(cfg, x)
    in_maps = []
    for ci in range(c.n_cores):
        in_maps.append({"xp": xs[ci],
                        "ppk": pack[ci * pr:(ci + 1) * pr]})
    return in_maps


def make_concat_inputs(cfg: Cfg, x, **params):
    """Axis-0-concatenated global inputs (cached-jit fast path)."""
    c = cfg
    n = c.n_cores
    xs = _x_timemajor(cfg, x)
    return {"ppk": _param_pack(cfg, **params),
            "xp": xs.reshape(n * c.J, c.T0 + c.PADT, c.B_loc)}


_CACHE = {}


def _get_nc(cfg: Cfg):
    key = ("nc", cfg.T0, cfg.B_loc, cfg.J, cfg.H, cfg.O, cfg.K, cfg.n_cores,
           cfg.mm_bf16)
    if key not in _CACHE:
        _CACHE[key] = build_kernel(cfg)
    return _CACHE[key]


class _Runner:
    """Cached PJRT executor: jit(shard_map(bass_exec)) built once, reused
    across kernel() calls. Mirrors bass2jax.run_bass_via_pjrt."""

    def __init__(self, cfg: Cfg):
        import jax
        from jax.sharding import Mesh, PartitionSpec
        try:
            from jax.experimental.shard_map import shard_map
        except ImportError:
            from jax.shard_map import shard_map
        from concourse import bass2jax

        self.cfg = cfg
        self.jax = jax
        nc = _get_nc(cfg)
        bass2jax.install_neuronx_cc_hook()
        partition_name = (nc.partition_id_tensor.name
                          if nc.partition_id_tensor else None)
        in_names, out_names, out_avals, zero_shapes = [], [], [], []
        for alloc in nc.m.functions[0].allocations:
            if not isinstance(alloc, mybir.MemoryLocationSet):
                continue
            name = alloc.memorylocations[0].name
            if alloc.kind == "ExternalInput":
                if name != partition_name:
                    in_names.append(name)
            elif alloc.kind == "ExternalOutput":
                out_names.append(name)
                shape = tuple(alloc.tensor_shape)
                dtype = mybir.dt.np(alloc.dtype)
                out_avals.append(jax.core.ShapedArray(shape, dtype))
                zero_shapes.append((shape, dtype))
        n_params = len(in_names)
        all_names = in_names + out_names + (
            [partition_name] if partition_name else [])
        donate = tuple(range(n_params, n_params + len(out_names)))
        self.in_names = in_names
        self.out_names = out_names
        self.zero_shapes = zero_shapes

        def _body(*args):
            operands = list(args)
            if partition_name is not None:
                operands.append(bass2jax.partition_id_tensor())
            outs = bass2jax._bass_exec_p.bind(
                *operands, out_avals=tuple(out_avals),
                in_names=tuple(all_names), out_names=tuple(out_names),
                lowering_input_output_aliases=(), sim_require_finite=True,
                sim_require_nnan=True, nc=nc)
            return tuple(outs)

        devices = jax.devices()[:cfg.n_cores]
        assert len(devices) == cfg.n_cores
        mesh = Mesh(np.asarray(devices), ("core",))
        in_specs = (PartitionSpec("core"),) * (n_params + len(out_names))
        out_specs = (PartitionSpec("core"),) * len(out_names)
        self.fn = jax.jit(
            shard_map(_body, mesh=mesh, in_specs=in_specs,
                      out_specs=out_specs, check_rep=False),
            donate_argnums=donate, keep_unused=True)

    def __call__(self, concat_inputs):
        n = self.cfg.n_cores
        args = [concat_inputs[name] for name in self.in_names]
        args += [np.zeros((n * s[0], *s[1:]), dt)
                 for (s, dt) in self.zero_shapes]
        outs = self.fn(*args)
        return np.asarray(outs[self.out_names.index("out")])


def _get_runner(cfg: Cfg) -> _Runner:
    key = ("runner", cfg.T0, cfg.B_loc, cfg.J, cfg.H, cfg.O, cfg.K,
           cfg.n_cores, cfg.mm_bf16)
    if key not in _CACHE:
        _CACHE[key] = _Runner(cfg)
    return _CACHE[key]


def run(cfg: Cfg, inputs, trace=False):
    """Reference path through run_bass_kernel_spmd (uncached jit)."""
    nc = _get_nc(cfg)
    in_maps = make_in_maps(cfg, **inputs)
    res = run_bass_kernel_spmd(nc, in_maps, core_ids=list(range(cfg.n_cores)),
                               trace=trace)
    outs = [res.results[ci]["out"].reshape(cfg.B_loc, cfg.O)
            for ci in range(cfg.n_cores)]
    return np.concatenate(outs, axis=0), res


def run_fast(cfg: Cfg, inputs):
    r = _get_runner(cfg)
    ci = make_concat_inputs(cfg, **inputs)
    out = r(ci)
    return out.reshape(cfg.B_tot, cfg.O)


def kernel(**inputs):
    cfg = Cfg()
    return run_fast(cfg, inputs)


# revision 39
# speedup vs baseline: 41.8227x; 1.6495x over previous
"""Trainium2 Bass kernel for nn_DelayLIFSNN.

Architecture (per reference):
  x (B, T0, J) -> delay_conv(w0,p0) -> BN(global batch stats) -> LIF
               -> delay_conv(w1,p1) -> BN -> LIF
               -> delay_conv(wr,pr) -> LI readout -> sum_t softmax_o -> (B, O)

Sharding: data-parallel over batch B across 8 cores (B_loc=32/core);
weights replicated; BN stats all-reduced ((128, 2*HT) f32 = 4KB each).

Host->device traffic is the bottleneck (axon tunnel ~60 MB/s), so the
Dcls gaussian delay kernels are expanded ON DEVICE from the raw (w, p)
parameters (2.7 MB/core) instead of shipping the K=25-expanded conv
kernels (34 MB/core). The jitted PJRT executable is cached across
kernel() calls.

Conv = sum over K=25 taps of shifted matmuls accumulated in PSUM.
LIF = per-step scalar_tensor_tensor ops on DVE (sequential over time).
LI readout = tensor_tensor_scan. Softmax+time-sum via PE transpose + ones-matmul.

Activation layouts:
  x / spikes (conv rhs): [ch_tile][ch_part 128, t*B + b]   (DRAM: [CT,128,Tpad,B])
  conv out psum:         [out_part 128, t*B + b] per (ht, time-tile)
  y DRAM:                [HT, 128, T, B]
  LIF scan tiles:        [h_part 128, t*(HT*B) + ht*B + b]
  readout y3 DRAM:       [B*O, T3]  (rows b*O+o)
"""

import sys
import numpy as np

try:
    import concourse.bass as bass
except ImportError:  # grading env fallback
    sys.path.insert(0, "/opt/trn_rl_repo")
    import concourse.bass as bass

import concourse.mybir as mybir
import concourse.tile as tile
from contextlib import ExitStack
from concourse import bacc
from concourse.bass_utils import run_bass_kernel_spmd
from concourse.masks import make_identity

F32 = mybir.dt.float32
BF16 = mybir.dt.bfloat16
U16 = mybir.dt.uint16
U8 = mybir.dt.uint8
AF = mybir.ActivationFunctionType
OP = mybir.AluOpType


class Cfg:
    def __init__(self, T0=300, B_loc=32, J=140, H=512, O=20, K=25, n_cores=8,
                 BETA=0.95, THRESH=1.0, SIG=0.5, EPS=1e-5, NT=16, CH=48,
                 CHUNK_TT=6, mm_bf16=False, x_bits=16, dbg=False,
                 max_phase=9, ablate=()):
        self.T0, self.B_loc, self.J, self.H, self.O, self.K = T0, B_loc, J, H, O, K
        self.n_cores = n_cores
        self.BETA, self.THRESH, self.SIG, self.EPS = BETA, THRESH, SIG, EPS
        self.LPAD, self.RPAD = K - 1, (K - 1) // 2
        self.PADT = self.LPAD + self.RPAD                      # 36
        self.T1 = T0 + self.RPAD                               # 312
        self.T2 = self.T1 + self.RPAD                          # 324
        self.T3 = self.T2 + self.RPAD                          # 336
        self.NT = NT                                           # out-steps per matmul tile
        self.CH = CH                                           # LIF chunk steps
        self.CHUNK_TT = CHUNK_TT                               # time-tiles per psum chunk
        self.HT = (H + 127) // 128                             # h tiles (4)
        self.B_tot = B_loc * n_cores
        self.mm_bf16 = mm_bf16                                 # matmul operand dtype
        self.MMDT = BF16 if mm_bf16 else F32
        self.x_bits = x_bits              # ship x as uN fixed point (16/8/0=f32)
        self.x_u16 = x_bits in (8, 16)
        self.XDT = ({16: U16, 8: U8}.get(x_bits, self.MMDT))
        self.XSCALE = float(2 ** x_bits - 1) if self.x_u16 else 1.0
        self.dbg = dbg
        self.max_phase = max_phase
        self.ablate = set(ablate)
        self.J0 = min(J, 128)
        self.JL = J - self.J0                                  # leftover channels (12)


def split_tiles(total, size):
    out = []
    t = 0
    while t < total:
        n = min(size, total - t)
        out.append((t, n))
        t += n
    return out


def bc(ap, axis, count):
    """Insert a stride-0 (broadcast) axis at position `axis` of an AP."""
    dims = [list(d) for d in ap.ap]
    dims.insert(axis, [0, count])
    return bass.AP(tensor=ap.tensor, offset=ap.offset, ap=dims)


def param_rows(c: Cfg):
    """Row offsets of each packed param in the [R, H] param pack."""
    PC = c.H
    r = {}
    pos = 0
    for name, n in (("w0T", c.J * c.H), ("p0T", c.J * c.H),
                    ("w1T", c.H * c.H), ("p1T", c.H * c.H),
                    ("wrT", c.H * c.O), ("prT", c.H * c.O),
                    ("g0m", 128 * c.HT), ("b0m", 128 * c.HT),
                    ("g1m", 128 * c.HT), ("b1m", 128 * c.HT),
                    ("selb", 128 * c.B_loc)):
        assert n % PC == 0
        r[name] = pos
        pos += n // PC
    r["tot"] = pos
    r["pad"] = ((pos + c.n_cores - 1) // c.n_cores) * c.n_cores
    return r


def build_kernel(cfg: Cfg):
    c = cfg
    B, HT, K, H, O = c.B_loc, c.HT, c.K, c.H, c.O
    MMDT = c.MMDT
    nc = bacc.Bacc("TRN2", target_bir_lowering=False, debug=False,
                   num_devices=c.n_cores)

    tts1 = split_tiles(c.T1, c.NT)
    tts2 = split_tiles(c.T2, c.NT)
    tts3 = split_tiles(c.T3, c.NT)
    n1slots = len(tts1)
    n2slots = len(tts2)

    # ---- I/O ----
    # x, time-major transposed on host; time pads added on device
    xp = nc.dram_tensor("xp", [c.J, c.T0, B], c.XDT, kind="ExternalInput")
    # All replicated params ship as ONE per-core 1/8 slice of a packed
    # [R, PC] f32 buffer; an on-device AllGather reconstructs the full
    # pack on every core (saves 8x on the host->device tunnel).
    PC = H  # pack row width; all param sizes are multiples of H=512
    rows = param_rows(c)
    ppk = nc.dram_tensor("ppk", [rows["pad"] // c.n_cores, PC], F32,
                         kind="ExternalInput")
    out = nc.dram_tensor("out", [B, O], F32, kind="ExternalOutput")
    if c.dbg:
        d_y1 = nc.dram_tensor("d_y1", [HT, 128, c.T1, B], F32, kind="ExternalOutput")
        d_s1 = nc.dram_tensor("d_s1", [HT, 128, c.T1 + c.PADT, B], F32, kind="ExternalOutput")
        d_y2 = nc.dram_tensor("d_y2", [HT, 128, c.T2, B], F32, kind="ExternalOutput")
        d_y3 = nc.dram_tensor("d_y3", [O, c.T3, B], F32, kind="ExternalOutput")
        d_w = nc.dram_tensor("d_w", [K, c.J0, H], F32, kind="ExternalOutput")

    with tile.TileContext(nc) as tc, ExitStack() as ctx:
        dram = ctx.enter_context(tc.tile_pool(name="dram", bufs=1, space="DRAM"))
        # expanded Dcls kernels, produced on device
        w0a = dram.tile([K, c.J0, H], MMDT, name="w0a")
        w0b = dram.tile([K, c.JL, H], MMDT, name="w0b") if c.JL else None
        w1t = dram.tile([K, H, H], MMDT, name="w1t")
        wrt = dram.tile([K, H, O], MMDT, name="wrt")
        y1d = dram.tile([HT, 128, c.T1, B], F32, name="y1d")
        s1d = dram.tile([HT, 128, c.T1 + c.PADT, B], MMDT, name="s1d")
        y2d = dram.tile([HT, 128, c.T2, B], F32, name="y2d")
        s2d = dram.tile([HT, 128, c.T2 + c.PADT, B], MMDT, name="s2d")
        y3d = dram.tile([O, c.T3, B], F32, name="y3d")
        cc_space = "Shared" if c.n_cores > 4 else "Local"
        cc1i = dram.tile([128, 2 * HT], F32, name="cc1i")
        cc1o = dram.tile([128, 2 * HT], F32, name="cc1o", addr_space=cc_space)
        cc2i = dram.tile([128, 2 * HT], F32, name="cc2i")
        cc2o = dram.tile([128, 2 * HT], F32, name="cc2o", addr_space=cc_space)

        # gather the full param pack from the per-core 1/8 slices
        # (collectives can't touch IO tensors; bounce input via internal DRAM)
        ppi = dram.tile([rows["pad"] // c.n_cores, PC], F32, name="ppi")
        nc.sync.dma_start(out=ppi, in_=ppk.ap())
        ppg = dram.tile([rows["pad"], PC], F32, name="ppg",
                        addr_space=cc_space)
        nc.gpsimd.collective_compute(
            "AllGather", OP.bypass,
            replica_groups=[list(range(c.n_cores))],
            ins=[ppi], outs=[ppg])

        def pview(name, shape):
            """AP view of packed param `name` with 2-D `shape`."""
            nrows = (shape[0] * shape[1]) // PC
            v = ppg[rows[name]:rows[name] + nrows, :]
            if shape[1] == PC:
                return v
            return (v.rearrange("a b -> (a b)")
                     .rearrange("(i j) -> i j", j=shape[1]))

        glob = ctx.enter_context(tc.tile_pool(name="glob", bufs=1))

        # persistent small tiles
        sum1 = glob.tile([128, HT * n1slots], F32, name="sum1")
        sq1 = glob.tile([128, HT * n1slots], F32, name="sq1")
        sum2 = glob.tile([128, HT * n2slots], F32, name="sum2")
        sq2 = glob.tile([128, HT * n2slots], F32, name="sq2")
        gam0 = glob.tile([128, HT], F32, name="gam0")
        bet0 = glob.tile([128, HT], F32, name="bet0")
        gam1 = glob.tile([128, HT], F32, name="gam1")
        bet1 = glob.tile([128, HT], F32, name="bet1")
        if "nogb" not in c.ablate:
            nc.sync.dma_start(out=gam0, in_=pview("g0m", (128, HT)))
            nc.sync.dma_start(out=bet0, in_=pview("b0m", (128, HT)))
            nc.sync.dma_start(out=gam1, in_=pview("g1m", (128, HT)))
            nc.sync.dma_start(out=bet1, in_=pview("b1m", (128, HT)))
        A1 = glob.tile([128, HT], F32, name="A1")
        C1b = glob.tile([128, HT * B], F32, name="C1b")
        A2 = glob.tile([128, HT], F32, name="A2")
        C2b = glob.tile([128, HT * B], F32, name="C2b")
        zpad = glob.tile([128, c.LPAD * B], MMDT, name="zpad")
        nc.vector.memset(zpad, 0.0)

        # zero the pad regions of the spike dram buffers
        for sd, T in (() if "nozpad" in c.ablate else ((s1d, c.T1), (s2d, c.T2))):
            for ht in range(HT):
                nc.sync.dma_start(out=sd[ht, :, 0:c.LPAD, :],
                                  in_=zpad.rearrange("p (t b) -> p t b", b=B))
                nc.sync.dma_start(
                    out=sd[ht, :, T + c.LPAD:T + c.PADT, :],
                    in_=zpad.rearrange("p (t b) -> p t b", b=B)[:, :c.RPAD, :])

        # =============== Phase 0: expand Dcls kernels on device ===========
        # g_k = exp(-0.5*((k - K//2 - p)/SIG)^2), normalized over k, * w.
        # SIG=0.5 -> exp(-2*(p - (k - K//2))^2).
        def expand(wsrc, psrc, r0, rp, M, dst, dstsl, tag):
            """rows r0:r0+rp of wsrc/psrc ([R, M] dram) -> dst[k][dstsl, :M]."""
            with ExitStack() as pe:
                ep = pe.enter_context(tc.tile_pool(name=f"exp{tag}", bufs=1))
                wp = ep.tile([rp, M], F32, name=f"wp{tag}")
                pp = ep.tile([rp, M], F32, name=f"pp{tag}")
                nc.sync.dma_start(out=wp, in_=wsrc[r0:r0 + rp, :])
                nc.sync.dma_start(out=pp, in_=psrc[r0:r0 + rp, :])
                eb = ep.tile([rp, K * M], F32, name=f"eb{tag}")
                ss = ep.tile([rp, M], F32, name=f"ss{tag}")
                dk = ep.tile([rp, M], F32, name=f"dk{tag}")
                sq = ep.tile([rp, M], F32, name=f"sq{tag}")
                inv2s2 = -1.0 / (2.0 * c.SIG * c.SIG)
                for k in range(K):
                    ek = eb[:, k * M:(k + 1) * M]
                    nc.vector.tensor_scalar(dk, pp, float(k - K // 2), None,
                                            OP.subtract)
                    nc.scalar.activation(out=sq, in_=dk, func=AF.Square)
                    nc.scalar.activation(out=ek, in_=sq, func=AF.Exp,
                                         scale=float(inv2s2))
                    if k == 0:
                        nc.vector.tensor_copy(ss, ek)
                    else:
                        nc.vector.tensor_add(ss, ss, ek)
                nc.vector.tensor_scalar_add(ss, ss, 1e-7)
                rn = ep.tile([rp, M], F32, name=f"rn{tag}")
                nc.vector.reciprocal(rn, ss)
                wn = ep.tile([rp, M], F32, name=f"wn{tag}")
                nc.vector.tensor_mul(wn, wp, rn)
                ok = ep.tile([rp, K * M], MMDT, name=f"ok{tag}")
                for k in range(K):
                    nc.vector.tensor_mul(ok[:, k * M:(k + 1) * M],
                                         eb[:, k * M:(k + 1) * M], wn)
                for k in range(K):
                    nc.sync.dma_start(out=dst[k, dstsl, :],
                                      in_=ok[:, k * M:(k + 1) * M])

        w0v, p0v = pview("w0T", (c.J, H)), pview("p0T", (c.J, H))
        w1v, p1v = pview("w1T", (H, H)), pview("p1T", (H, H))
        wrv, prv = pview("wrT", (H, O)), pview("prT", (H, O))
        expand(w0v, p0v, 0, c.J0, H, w0a, slice(0, c.J0), "0a")
        if c.JL:
            expand(w0v, p0v, c.J0, c.JL, H, w0b, slice(0, c.JL), "0b")
        for ct in range(HT):
            expand(w1v, p1v, ct * 128, 128, H, w1t,
                   slice(ct * 128, (ct + 1) * 128), f"1_{ct}")
            expand(wrv, prv, ct * 128, 128, O, wrt,
                   slice(ct * 128, (ct + 1) * 128), f"r_{ct}")
        if c.dbg:
            nc.sync.dma_start(out=d_w.ap(), in_=w0a)

        # =============== Phase 1: conv1 (x -> y1) + stats ===============
        with ExitStack() as p1:
            psum = p1.enter_context(tc.tile_pool(name="psum1", bufs=8,
                                                  space="PSUM"))
            xpool = p1.enter_context(tc.tile_pool(name="xpool", bufs=1))

            T0p = c.T0 + c.PADT

            with ExitStack() as pxu:
                xup = (pxu.enter_context(tc.tile_pool(name="xup", bufs=1))
                       if c.x_u16 else None)

                def load_x(jp, j0):
                    """[jp, T0p*B] MMDT tile; pads zeroed and middle
                    dequantized, all on the ACT engine (single queue — no
                    cross-engine write ordering)."""
                    X = xpool.tile([jp, T0p * B], MMDT, name=f"X_{j0}")
                    lp, mid0 = c.LPAD * B, (c.LPAD + c.T0) * B
                    nc.scalar.activation(out=X[:, :lp], in_=zpad[:jp, :lp],
                                         func=AF.Copy)
                    nc.scalar.activation(out=X[:, mid0:],
                                         in_=zpad[:jp, :T0p * B - mid0],
                                         func=AF.Copy)
                    if c.x_u16:
                        XU = xup.tile([jp, c.T0 * B], c.XDT, name=f"XU_{j0}")
                        nc.sync.dma_start(
                            out=XU.rearrange("p (t b) -> p t b", b=B),
                            in_=xp.ap()[j0:j0 + jp])
                        nc.scalar.activation(out=X[:, lp:mid0], in_=XU,
                                             func=AF.Copy,
                                             scale=float(1.0 / c.XSCALE))
                    else:
                        nc.sync.dma_start(
                            out=X.rearrange("p (t b) -> p t b", b=B)[
                                :, c.LPAD:c.LPAD + c.T0, :],
                            in_=xp.ap()[j0:j0 + jp])
                    return X

                X0 = load_x(c.J0, 0)
                X1 = load_x(c.JL, c.J0) if c.JL else None

            wpool1 = p1.enter_context(tc.tile_pool(name="wpool1", bufs=1))
            stg1 = p1.enter_context(tc.tile_pool(name="stg1", bufs=3))
            W0 = wpool1.tile([c.J0, K * H], MMDT, name="W0")
            nc.sync.dma_start(out=W0.rearrange("p (k h) -> p k h", h=H),
                              in_=w0a.rearrange("k p h -> p k h"))
            if c.JL:
                W1l = wpool1.tile([c.JL, K * H], MMDT, name="W1l")
                nc.sync.dma_start(out=W1l.rearrange("p (k h) -> p k h", h=H),
                                  in_=w0b.rearrange("k p h -> p k h"))

            n_mm = K * (2 if c.JL else 1)
            zsrc = None
            if "nomm" in c.ablate:
                zsrc = wpool1.tile([128, c.NT * B], F32, name="zsrc")
                nc.vector.memset(zsrc, 0.0)
            for tti, (t0, nt) in enumerate(tts1):
                for ht in range(HT):
                    if "nomm" not in c.ablate:
                        ps = psum.tile([128, nt * B], F32, tag="cv1ps", name="ps1")
                        mi = 0
                        for kk in range(K):
                            nc.tensor.matmul(
                                ps, lhsT=W0[:, kk * H + ht * 128: kk * H + ht * 128 + 128],
                                rhs=X0[:, (t0 + kk) * B:(t0 + kk) * B + nt * B],
                                start=(mi == 0), stop=(mi == n_mm - 1))
                            mi += 1
                            if c.JL:
                                nc.tensor.matmul(
                                    ps,
                                    lhsT=W1l[:, kk * H + ht * 128: kk * H + ht * 128 + 128],
                                    rhs=X1[:, (t0 + kk) * B:(t0 + kk) * B + nt * B],
                                    start=(mi == 0), stop=(mi == n_mm - 1))
                                mi += 1
                        src = ps
                    else:
                        src = zsrc[:, :nt * B]
                    slot = ht * n1slots + tti
                    ystg = stg1.tile([128, nt * B], F32, tag="ystg", name="ystg")
                    if "nostats" not in c.ablate:
                        nc.scalar.activation(out=ystg, in_=src, func=AF.Copy,
                                             accum_out=sum1[:, slot:slot + 1])
                        ysq = stg1.tile([128, nt * B], F32, tag="ysq", name="ysq")
                        nc.scalar.activation(out=ysq, in_=src, func=AF.Square,
                                             accum_out=sq1[:, slot:slot + 1])
                    else:
                        nc.scalar.activation(out=ystg, in_=src, func=AF.Copy)
                    if "nostore" not in c.ablate:
                        nc.sync.dma_start(
                            out=y1d[ht, :, t0:t0 + nt, :],
                            in_=ystg.rearrange("p (t b) -> p t b", b=B))

        # =============== BN stats: allreduce + affine ===============
        def bn_affine(sumt, sqt, nslots, N, gam, bet, cci, cco, A, Cb, tagp):
            with ExitStack() as pb:
                sp = pb.enter_context(tc.tile_pool(name=f"bn{tagp}", bufs=1))
                ccs = sp.tile([128, 2 * HT], F32, name=f"ccs{tagp}")
                nc.vector.reduce_sum(
                    out=ccs[:, 0:HT],
                    in_=sumt.rearrange("p (h s) -> p h s", s=nslots),
                    axis=mybir.AxisListType.X)
                nc.vector.reduce_sum(
                    out=ccs[:, HT:2 * HT],
                    in_=sqt.rearrange("p (h s) -> p h s", s=nslots),
                    axis=mybir.AxisListType.X)
                nc.sync.dma_start(out=cci, in_=ccs)
                nc.gpsimd.collective_compute(
                    "AllReduce", OP.add,
                    replica_groups=[list(range(c.n_cores))],
                    ins=[cci], outs=[cco])
                gs = sp.tile([128, 2 * HT], F32, name=f"gs{tagp}")
                nc.sync.dma_start(out=gs, in_=cco)
                rN = float(1.0 / N)
                mu = sp.tile([128, HT], F32, name=f"mu{tagp}")
                nc.vector.tensor_scalar(mu, gs[:, 0:HT], rN, None, OP.mult)
                ex2 = sp.tile([128, HT], F32, name=f"ex2{tagp}")
                nc.vector.tensor_scalar(ex2, gs[:, HT:2 * HT], rN, None,
                                        OP.mult)
                var = sp.tile([128, HT], F32, name=f"var{tagp}")
                # var = ex2 - mu*mu ; then + eps
                nc.vector.scalar_tensor_tensor(out=var, in0=mu, scalar=1.0,
                                               in1=mu, op0=OP.mult, op1=OP.mult)
                nc.vector.tensor_sub(var, ex2, var)
                nc.vector.tensor_scalar_add(var, var, float(c.EPS))
                sv = sp.tile([128, HT], F32, name=f"sv{tagp}")
                nc.scalar.activation(out=sv, in_=var, func=AF.Sqrt)
                # one Newton step: s' = 0.5*(s + v/s)  (ACT sqrt is ~3e-6 approx)
                rs0 = sp.tile([128, HT], F32, name=f"rs0{tagp}")
                nc.vector.reciprocal(rs0, sv)
                t1 = sp.tile([128, HT], F32, name=f"t1{tagp}")
                nc.vector.tensor_mul(t1, var, rs0)
                nc.vector.tensor_add(sv, sv, t1)
                nc.vector.tensor_scalar(sv, sv, 0.5, None, OP.mult)
                rsv = sp.tile([128, HT], F32, name=f"rsv{tagp}")
                nc.vector.reciprocal(rsv, sv)
                nc.vector.tensor_mul(A, gam, rsv)
                # Cbias = bet - mu*A, broadcast over batch
                cb1 = sp.tile([128, HT], F32, name=f"cb1{tagp}")
                nc.vector.tensor_mul(cb1, mu, A)
                nc.vector.tensor_sub(cb1, bet, cb1)
                nc.vector.tensor_copy(
                    Cb.rearrange("p (h b) -> p h b", b=B), bc(cb1, 2, B))

        if c.max_phase >= 2:
            bn_affine(sum1, sq1, n1slots, c.T1 * c.B_tot, gam0, bet0,
                      cc1i, cc1o, A1, C1b, "1")

        # =============== LIF layer (generic) ===============
        def lif_layer(yd, sd, A, Cb, T, tag):
            with ExitStack() as pl:
                lp = pl.enter_context(tc.tile_pool(name=f"lif{tag}", bufs=2))
                up = pl.enter_context(tc.tile_pool(name=f"lifu{tag}", bufs=1))
                HTB = HT * B
                U = up.tile([128, HTB], F32, name=f"U{tag}")
                nc.vector.memset(U, 0.0)
                for (c0, cn) in split_tiles(T, c.CH):
                    ybufs = []
                    for ht in range(HT):
                        yb = lp.tile([128, cn * B], F32, tag=f"yb{ht}",
                                     name=f"yb{tag}")
                        nc.sync.dma_start(
                            out=yb.rearrange("p (t b) -> p t b", b=B),
                            in_=yd[ht, :, c0:c0 + cn, :])
                        ybufs.append(yb)
                    scn = lp.tile([128, cn * HTB], F32, tag="scn",
                                  name=f"scn{tag}")
                    scn3 = scn.rearrange("p (t x) -> p t x", x=HTB)
                    for ht in range(HT):
                        nc.vector.scalar_tensor_tensor(
                            out=scn3[:, :, ht * B:(ht + 1) * B],
                            in0=ybufs[ht].rearrange("p (t b) -> p t b", b=B),
                            scalar=A[:, ht:ht + 1],
                            in1=bc(Cb[:, ht * B:(ht + 1) * B], 1, cn),
                            op0=OP.mult, op1=OP.add)
                    S = lp.tile([128, cn * HTB], MMDT, tag="S", name=f"S{tag}")
                    for t in range(cn):
                        sl = slice(t * HTB, (t + 1) * HTB)
                        ut = lp.tile([128, HTB], F32, tag="ut", name=f"ut{tag}")
                        nc.vector.scalar_tensor_tensor(
                            out=ut, in0=U, scalar=float(c.BETA),
                            in1=scn[:, sl], op0=OP.mult, op1=OP.add)
                        nc.vector.tensor_scalar(
                            S[:, sl], ut, float(c.THRESH), None, OP.is_ge)
                        nc.vector.scalar_tensor_tensor(
                            out=U, in0=ut, scalar=float(c.THRESH), in1=ut,
                            op0=OP.is_lt, op1=OP.mult)
                    S3 = S.rearrange("p (t h b) -> p t h b", h=HT, b=B)
                    for ht in range(HT):
                        nc.sync.dma_start(
                            out=sd[ht, :, c.LPAD + c0:c.LPAD + c0 + cn, :],
                            in_=S3[:, :, ht, :])

        if c.max_phase >= 3:
            lif_layer(y1d, s1d, A1, C1b, c.T1, "1")

        # =============== conv from spikes (generic: layer 2 & readout) =====
        def conv_sp(sd, wsrc, M, tts, yd=None, sumt=None, sqt=None,
                    nslots=0, y3=None, tag=""):
            """y[o, t] = sum_{ct,k} W_k[ct]^T s[ct, t+k] (padded s)."""
            MT = (M + 127) // 128
            tchunks = split_tiles(len(tts), c.CHUNK_TT)
            with ExitStack() as pc:
                psum = pc.enter_context(tc.tile_pool(name=f"psum{tag}",
                                                     bufs=8, space="PSUM"))
                swp = pc.enter_context(tc.tile_pool(name=f"swin{tag}", bufs=2))
                wp = pc.enter_context(tc.tile_pool(name=f"w{tag}", bufs=3))
                sg = pc.enter_context(tc.tile_pool(name=f"stg{tag}", bufs=3))
                for (tci, ntt) in tchunks:
                    tt_group = tts[tci:tci + ntt]
                    w0_ = tt_group[0][0]
                    last_t0, last_nt = tt_group[-1]
                    winlen = (last_t0 + last_nt - 1 + K - 1) - w0_ + 1
                    swin = []
                    for ct in range(HT):
                        sw = swp.tile([128, winlen * B], MMDT, tag=f"sw{ct}",
                                      name=f"sw{tag}")
                        nc.sync.dma_start(
                            out=sw.rearrange("p (t b) -> p t b", b=B),
                            in_=sd[ct, :, w0_:w0_ + winlen, :])
                        swin.append(sw)
                    for ht in range(MT):
                        m0 = ht * 128
                        mtw = min(128, M - m0)
                        pss = [psum.tile([128, nt * B], F32, tag="cvps",
                                         name=f"ps{tag}")
                               for (t0, nt) in tt_group]
                        n_acc = HT * K
                        mi = 0
                        for ct in range(HT):
                            wt = wp.tile([128, K * mtw], MMDT, tag="wt",
                                         name=f"wt{tag}")
                            nc.sync.dma_start(
                                out=wt.rearrange("p (k m) -> p k m", m=mtw),
                                in_=wsrc[:, ct * 128:(ct + 1) * 128,
                                         m0:m0 + mtw].rearrange(
                                             "k p m -> p k m"))
                            for kk in range(K):
                                lhsT = wt[:, kk * mtw:(kk + 1) * mtw]
                                st = (mi == 0)
                                sp_ = (mi == n_acc - 1)
                                for ti, (t0, nt) in enumerate(tt_group):
                                    off = (t0 - w0_ + kk) * B
                                    nc.tensor.matmul(
                                        pss[ti][:mtw], lhsT=lhsT,
                                        rhs=swin[ct][:, off:off + nt * B],
                                        start=st, stop=sp_)
                                mi += 1
                        for ti, (t0, nt) in enumerate(tt_group):
                            stg = sg.tile([128, nt * B], F32, tag="stg",
                                          name=f"stg{tag}")
                            if sumt is not None:
                                slot = ht * nslots + tci + ti
                                nc.scalar.activation(
                                    out=stg[:mtw], in_=pss[ti][:mtw],
                                    func=AF.Copy,
                                    accum_out=sumt[:, slot:slot + 1])
                                sqg = sg.tile([128, nt * B], F32, tag="sqg",
                                              name=f"sqg{tag}")
                                nc.scalar.activation(
                                    out=sqg[:mtw], in_=pss[ti][:mtw],
                                    func=AF.Square,
                                    accum_out=sqt[:, slot:slot + 1])
                            else:
                                nc.scalar.activation(out=stg[:mtw],
                                                     in_=pss[ti][:mtw],
                                                     func=AF.Copy)
                            if yd is not None:
                                nc.sync.dma_start(
                                    out=yd[ht, :, t0:t0 + nt, :],
                                    in_=stg.rearrange("p (t b) -> p t b", b=B))
                            else:  # readout: y3 is [O, T3, B]
                                nc.sync.dma_start(
                                    out=y3[m0:m0 + mtw, t0:t0 + nt, :],
                                    in_=stg[:mtw].rearrange(
                                        "p (t b) -> p t b", b=B))

        if c.max_phase >= 4:
            conv_sp(s1d, w1t, H, tts2, yd=y2d, sumt=sum2, sqt=sq2,
                    nslots=n2slots, tag="c2")
        if c.max_phase >= 5:
            bn_affine(sum2, sq2, n2slots, c.T2 * c.B_tot, gam1, bet1,
                      cc2i, cc2o, A2, C2b, "2")
        if c.max_phase >= 6:
            lif_layer(y2d, s2d, A2, C2b, c.T2, "2")
        if c.max_phase >= 7:
            conv_sp(s2d, wrt, O, tts3, y3=y3d, tag="c3")

        # =============== tail: LI scan, softmax over O, sum over t =========
        if c.max_phase < 8:
            with ExitStack() as pt:
                tp0 = pt.enter_context(tc.tile_pool(name="tail0", bufs=1))
                z = tp0.tile([B, O], F32, name="z")
                nc.vector.memset(z, 0.0)
                nc.sync.dma_start(out=out.ap(), in_=z)
        if c.max_phase >= 8:
            with ExitStack() as pt:
              psum = pt.enter_context(tc.tile_pool(name="psumt", bufs=1,
                                                   space="PSUM"))
              tp = pt.enter_context(tc.tile_pool(name="tail", bufs=1))
              tp2 = pt.enter_context(tc.tile_pool(name="tail2", bufs=3))
              TB = c.T3 * B
              Y3 = tp.tile([O, TB], F32, name="Y3")
              nc.sync.dma_start(out=Y3.rearrange("p (t b) -> p t b", b=B),
                                in_=y3d)
              beta_t = tp.tile([128, c.T3], F32, name="beta_t")
              nc.vector.memset(beta_t, float(c.BETA))
              idn = tp.tile([128, 128], F32, name="idn")
              make_identity(nc, idn)
              selbt = tp.tile([128, B], F32, name="selbt")
              nc.sync.dma_start(out=selbt, in_=pview("selb", (128, B)))
              us = tp.tile([O, TB], F32, name="us")
              # LI scan over t, one strided scan per batch column
              usv = us.rearrange("p (t b) -> p b t", b=B)
              y3v = Y3.rearrange("p (t b) -> p b t", b=B)
              for b in range(B):
                  nc.vector.tensor_tensor_scan(
                      out=usv[:, b, :], data0=beta_t[:O], data1=y3v[:, b, :],
                      initial=0.0, op0=OP.mult, op1=OP.add)
              # per-128-col blocks: transpose to (t*b, o), softmax over o, then
              # sum over t via selector matmul into (B, O)
              acc = psum.tile([B, O], F32, tag="accps", name="accps", bufs=1)
              blocks = split_tiles(TB, 128)
              for bi, (c0, cw) in enumerate(blocks):
                  pst = psum.tile([128, O], F32, tag="tpps", name="tpps", bufs=2)
                  nc.tensor.transpose(out=pst[:cw, :O],
                                      in_=us[:, c0:c0 + cw],
                                      identity=idn[:O, :O])
                  v = tp2.tile([128, O], F32, tag="v", name="v")
                  nc.scalar.copy(out=v[:cw], in_=pst[:cw, :O])
                  mx = tp2.tile([128, 1], F32, tag="mx", name="mx")
                  nc.vector.reduce_max(out=mx[:cw], in_=v[:cw],
                                       axis=mybir.AxisListType.X)
                  ev = tp2.tile([128, O], F32, tag="ev", name="ev")
                  nc.vector.tensor_scalar(ev[:cw], v[:cw], mx[:cw], None,
                                          OP.subtract)
                  pv = tp2.tile([128, O], F32, tag="pv", name="pv")
                  sm = tp2.tile([128, 1], F32, tag="sm", name="sm")
                  nc.scalar.activation(out=pv[:cw], in_=ev[:cw], func=AF.Exp,
                                       accum_out=sm[:cw])
                  rsm = tp2.tile([128, 1], F32, tag="rsm", name="rsm")
                  nc.vector.reciprocal(rsm[:cw], sm[:cw])
                  pn_t = tp2.tile([128, O], F32, tag="pnt", name="pnt")
                  nc.vector.tensor_scalar(pn_t[:cw], pv[:cw], rsm[:cw], None,
                                          OP.mult)
                  nc.tensor.matmul(
                      acc, lhsT=selbt[:cw], rhs=pn_t[:cw],
                      start=(bi == 0), stop=(bi == len(blocks) - 1),
                      skip_group_check=True)
              res = tp.tile([B, O], F32, name="res")
              nc.scalar.copy(out=res, in_=acc)
              nc.sync.dma_start(out=out.ap(), in_=res)
        if c.dbg:
            nc.sync.dma_start(out=d_y1.ap(), in_=y1d)
            nc.sync.dma_start(out=d_s1.ap(), in_=s1d)
            nc.sync.dma_start(out=d_y2.ap(), in_=y2d)
            nc.sync.dma_start(out=d_y3.ap(), in_=y3d)

    nc.compile()
    return nc


# ======================= host side =======================

def dcls_np(w, p, K, SIG):
    w = np.asarray(w, np.float32)
    p = np.asarray(p, np.float32)
    idx = np.arange(K, dtype=np.float32)
    d = idx[None, None, :] - np.float32(K // 2) - p[:, :, None]
    t = d / np.float32(SIG)
    g = np.exp(np.float32(-0.5) * t * t).astype(np.float32)
    g = g / (np.sum(g, axis=-1, keepdims=True, dtype=np.float32)
             + np.float32(1e-7))
    return (w[:, :, None] * g).astype(np.float32)


def _np_dt(c):
    return mybir.dt.np(c.MMDT)


def _param_pack(cfg: Cfg, w0, p0, g0, b0, w1, p1, g1, b1, wr, pr):
    """Packed [R_pad, H] f32 param buffer (order must match param_rows)."""
    c = cfg
    PC = c.H

    def chanmat(v):
        return np.ascontiguousarray(
            np.asarray(v, np.float32).reshape(c.HT, 128).T)

    def T(a):
        return np.ascontiguousarray(np.asarray(a, np.float32).T)

    nrep = max(1, (128 + c.B_loc - 1) // c.B_loc)
    selb = np.ascontiguousarray(
        np.tile(np.eye(c.B_loc, dtype=np.float32), (nrep, 1)))[:128]
    parts = [T(w0), T(p0), T(w1), T(p1), T(wr), T(pr),
             chanmat(g0), chanmat(b0), chanmat(g1), chanmat(b1), selb]
    rows = param_rows(c)
    flat = np.zeros(rows["pad"] * PC, np.float32)
    pos = 0
    for p in parts:
        n = p.size
        flat[pos:pos + n] = np.ascontiguousarray(p, np.float32).reshape(-1)
        pos += n
    assert pos == rows["tot"] * PC
    return flat.reshape(rows["pad"], PC)


def _x_timemajor(cfg: Cfg, x):
    """(B_tot, T0, J) -> (n_cores, J, T0+PADT, B_loc) zero-padded, wire dtype."""
    from concurrent.futures import ThreadPoolExecutor
    c = cfg
    x = np.asarray(x, np.float32)
    xr = x.reshape(c.n_cores, c.B_loc, c.T0, c.J)
    qdt = np.uint16 if c.x_bits == 16 else (
        np.uint8 if c.x_bits == 8 else _np_dt(c))
    out = np.empty((c.n_cores, c.J, c.T0, c.B_loc), qdt)

    def work(ci):
        if c.x_u16:
            q = (xr[ci] * np.float32(c.XSCALE)
                 + np.float32(0.5)).astype(qdt)      # x in [0,1): no clip
        else:
            q = xr[ci]
        out[ci] = q.transpose(2, 1, 0)

    if c.n_cores > 1:
        with ThreadPoolExecutor(max_workers=8) as ex:
            list(ex.map(work, range(c.n_cores)))
    else:
        work(0)
    return out


def make_in_maps(cfg: Cfg, x, **params):
    """Per-core input dicts (sim / run_bass_kernel_spmd path)."""
    c = cfg
    pack = _param_pack(cfg, **params)
    pr = pack.shape[0] // c.n_cores
    xs = _x_timemajor(cfg, x)
    in_maps = []
    for ci in range(c.n_cores):
        in_maps.append({"xp": xs[ci],
                        "ppk": pack[ci * pr:(ci + 1) * pr]})
    return in_maps


def make_concat_inputs(cfg: Cfg, x, **params):
    """Axis-0-concatenated global inputs (cached-jit fast path)."""
    c = cfg
    n = c.n_cores
    xs = _x_timemajor(cfg, x)
    return {"ppk": _param_pack(cfg, **params),
            "xp": xs.reshape(n * c.J, c.T0, c.B_loc)}


_CACHE = {}


def _get_nc(cfg: Cfg):
    key = ("nc", cfg.T0, cfg.B_loc, cfg.J, cfg.H, cfg.O, cfg.K, cfg.n_cores,
           cfg.mm_bf16, cfg.x_bits)
    if key not in _CACHE:
        _CACHE[key] = build_kernel(cfg)
    return _CACHE[key]


class _Runner:
    """Cached PJRT executor: jit(shard_map(bass_exec)) built once, reused
    across kernel() calls. Mirrors bass2jax.run_bass_via_pjrt."""

    def __init__(self, cfg: Cfg):
        import jax
        from jax.sharding import Mesh, PartitionSpec
        try:
            from jax.experimental.shard_map import shard_map
        except ImportError:
            from jax.shard_map import shard_map
        from concourse import bass2jax

        self.cfg = cfg
        self.jax = jax
        nc = _get_nc(cfg)
        bass2jax.install_neuronx_cc_hook()
        partition_name = (nc.partition_id_tensor.name
                          if nc.partition_id_tensor else None)
        in_names, out_names, out_avals, zero_shapes = [], [], [], []
        for alloc in nc.m.functions[0].allocations:
            if not isinstance(alloc, mybir.MemoryLocationSet):
                continue
            name = alloc.memorylocations[0].name
            if alloc.kind == "ExternalInput":
                if name != partition_name:
                    in_names.append(name)
            elif alloc.kind == "ExternalOutput":
                out_names.append(name)
                shape = tuple(alloc.tensor_shape)
                dtype = mybir.dt.np(alloc.dtype)
                out_avals.append(jax.core.ShapedArray(shape, dtype))
                zero_shapes.append((shape, dtype))
        n_params = len(in_names)
        all_names = in_names + out_names + (
            [partition_name] if partition_name else [])
        donate = tuple(range(n_params, n_params + len(out_names)))
        self.in_names = in_names
        self.out_names = out_names
        self.zero_shapes = zero_shapes

        def _body(*args):
            operands = list(args)
            if partition_name is not None:
                operands.append(bass2jax.partition_id_tensor())
            outs = bass2jax._bass_exec_p.bind(
                *operands, out_avals=tuple(out_avals),
                in_names=tuple(all_names), out_names=tuple(out_names),
                lowering_input_output_aliases=(), sim_require_finite=True,
                sim_require_nnan=True, nc=nc)
            return tuple(outs)

        devices = jax.devices()[:cfg.n_cores]
        assert len(devices) == cfg.n_cores
        mesh = Mesh(np.asarray(devices), ("core",))
        in_specs = (PartitionSpec("core"),) * (n_params + len(out_names))
        out_specs = (PartitionSpec("core"),) * len(out_names)
        self.fn = jax.jit(
            shard_map(_body, mesh=mesh, in_specs=in_specs,
                      out_specs=out_specs, check_rep=False),
            donate_argnums=donate, keep_unused=True)

    def __call__(self, concat_inputs):
        n = self.cfg.n_cores
        args = [concat_inputs[name] for name in self.in_names]
        args += [np.zeros((n * s[0], *s[1:]), dt)
                 for (s, dt) in self.zero_shapes]
        outs = self.fn(*args)
        return np.asarray(outs[self.out_names.index("out")])


def _get_runner(cfg: Cfg) -> _Runner:
    key = ("runner", cfg.T0, cfg.B_loc, cfg.J, cfg.H, cfg.O, cfg.K,
           cfg.n_cores, cfg.mm_bf16, cfg.x_bits)
    if key not in _CACHE:
        _CACHE[key] = _Runner(cfg)
    return _CACHE[key]


def run(cfg: Cfg, inputs, trace=False):
    """Reference path through run_bass_kernel_spmd (uncached jit)."""
    nc = _get_nc(cfg)
    in_maps = make_in_maps(cfg, **inputs)
    res = run_bass_kernel_spmd(nc, in_maps, core_ids=list(range(cfg.n_cores)),
                               trace=trace)
    outs = [res.results[ci]["out"].reshape(cfg.B_loc, cfg.O)
            for ci in range(cfg.n_cores)]
    return np.concatenate(outs, axis=0), res


def run_fast(cfg: Cfg, inputs):
    r = _get_runner(cfg)
    ci = make_concat_inputs(cfg, **inputs)
    out = r(ci)
    return out.reshape(cfg.B_tot, cfg.O)


def kernel(**inputs):
    cfg = Cfg()
    return run_fast(cfg, inputs)
